# revision 12
# baseline (speedup 1.0000x reference)
"""CrossAttention Trainium2 kernel (8 NeuronCores).

Reference computation (B=2, N=M=2048, D=1024, H=16, C=64):
    q = rmsnorm(querys @ Wq.T, gq) * C**-0.5       [B,N,D]
    k = rmsnorm(key_feats @ Wk.T, gk)              [B,M,D]
    v = key_feats @ Wv.T                           [B,M,D]
    attn = softmax(mask(q @ k.T per head))         [B,H,N,M]
    out = (attn @ v per head, concat) @ Wo.T + bo  [B,N,D]

Sharding: core = b*4 + j (b in {0,1}; j in {0..3} owns heads 4j..4j+3 = a
256-wide slice of D). Host pre-transposes inputs/weights, folds gq*scale /
gk into Wq / Wk rows, and pre-rounds everything to f32r (fp32 with 11-bit
mantissa -> full PE rate).

v2 structural changes vs v1:
  - Mask compaction: rows with mask==0 contribute exp(-inf)=0 to both the
    softmax denominator and PV, so the host gathers only the valid kf
    columns (per batch), pads to a multiple of 128, and the kernel runs
    with M_pad ~= 1152 instead of 2048.  All M-side work (k/v projection,
    QK, exp, PV, kf DMA) shrinks ~2x.  Padding columns carry bias -1e30
    into the exp -> contribute exactly 0.
  - k and v projections fused over a single kfT stream (halves kf DMA).
  - One fused AllReduce carries both q and k partial sum-of-squares
    (2048 + M_pad floats); its ~27us mesh latency is bridged by a long
    dependency-free dummy-matmul burst that keeps the PE HAM clock-gate
    warm (K=8/8) into attention.
  - Attention is software-pipelined depth-2 per (nbp, h) pass: PE order is
    ... PV(mt-1), QK(mt+1), filler, PV(mt) ... so the PE never stalls on
    the ACT exp (v1 stalled ~0.4us every mt, which kept HAM at K=4/8 =
    1.2 GHz for the whole 314us attention phase).  ACT exp (~1.2us/mt) is
    the pace-setter; PE real work is ~1.0us/mt, padded by a dummy matmul
    (first n-half) or an out-projection matmul pair (second n-half).
  - The out projection (partial over this core's d-slice) is interleaved
    into attention as filler work; the host sums 4 partials per b and
    adds bo.
"""

import numpy as np

import concourse.tile as tile
from concourse import bacc, mybir
from concourse.bass_utils import run_bass_kernel_spmd

B, N, M_FULL, D, H = 2, 2048, 2048, 1024, 16
C = D // H  # 64, head dim
E = D  # input feature dim
EPS = 1e-6
SCALE = C ** (-0.5)
DS = D // 4  # 256, per-core d-slice
NCORES = 8

f32 = mybir.dt.float32
f32r = mybir.dt.float32r
AF = mybir.ActivationFunctionType

NEG = -1e30
WARM_MM = 150  # dummy matmuls bridging the AllReduce latency (~42us @ 2.4GHz)


def round_f32r(x: np.ndarray) -> np.ndarray:
    b = np.ascontiguousarray(x, dtype=np.float32).view(np.uint32)
    b = (b + 0x800) & np.uint32(0xFFFFF000)
    return b.view(np.float32)


def build(n_mt: int):
    M = n_mt * 128
    mblocks = []
    off = 0
    while off < M:
        w = min(512, M - off)
        mblocks.append((off, w))
        off += w

    nc = bacc.Bacc(None, target_bir_lowering=False)

    qT_d = nc.declare_dram_parameter("qT", [E, N], f32r, isOutput=False)
    kfT_d = nc.declare_dram_parameter("kfT", [E, M], f32r, isOutput=False)
    wqT_d = nc.declare_dram_parameter("wqT", [E, DS], f32r, isOutput=False)
    wkT_d = nc.declare_dram_parameter("wkT", [E, DS], f32r, isOutput=False)
    wvT_d = nc.declare_dram_parameter("wvT", [E, DS], f32r, isOutput=False)
    woT_d = nc.declare_dram_parameter("woT", [DS, D], f32r, isOutput=False)
    ig2q_d = nc.declare_dram_parameter("ig2q", [2, 128], f32r, isOutput=False)
    ig2k_d = nc.declare_dram_parameter("ig2k", [2, 128], f32r, isOutput=False)
    mb_d = nc.declare_dram_parameter("mbias", [n_mt, 128], f32, isOutput=False)
    outT_d = nc.declare_dram_parameter("outT", [D, N], f32, isOutput=True)

    with (
        nc.allow_low_precision(reason="f32r matmul operands by design; fp32 PSUM"),
        tile.TileContext(nc) as tc,
    ):
        with (
            tc.tile_pool(name="singles", bufs=1) as singles,
            tc.tile_pool(name="wts", bufs=3) as wts,
            tc.tile_pool(name="blk", bufs=2) as blkpool,
            tc.tile_pool(name="sq", bufs=2) as sqpool,
            tc.tile_pool(name="psb", bufs=3) as ppool,
            tc.tile_pool(name="obuf", bufs=2) as obuf,
            tc.tile_pool(name="osb2", bufs=2) as outbuf,
            tc.tile_pool(name="rdp", bufs=6) as rdp,
            tc.tile_pool(name="small", bufs=2) as small,
            tc.tile_pool(name="dram", bufs=1, space="DRAM") as dram,
        ):
            # ---- constants / small inputs ----
            ones1x64 = singles.tile([1, 64], f32)
            nc.vector.memset(ones1x64, 1.0)
            ones1x128 = singles.tile([1, 128], f32)
            nc.vector.memset(ones1x128, 1.0)
            onesv = singles.tile([128, n_mt * 4], f32)
            nc.vector.memset(onesv, 1.0)
            eps_t = singles.tile([128, 1], f32)
            nc.vector.memset(eps_t, EPS)
            invd_t = singles.tile([128, 1], f32)
            nc.vector.memset(invd_t, 1.0 / D)
            ig2q_sb = singles.tile([128, 2], f32r)
            nc.sync.dma_start(out=ig2q_sb, in_=ig2q_d.rearrange("t p -> p t"))
            ig2k_sb = singles.tile([128, 2], f32r)
            nc.sync.dma_start(out=ig2k_sb, in_=ig2k_d.rearrange("t p -> p t"))
            mb_sb = singles.tile([128, n_mt], f32)
            nc.sync.dma_start(out=mb_sb, in_=mb_d.rearrange("t p -> p t"))

            # weights: wq, wk, wv upfront; wo reuses wq's slot after q proj
            wq_sb = wts.tile([128, 8, DS], f32r, tag="w")
            wk_sb = wts.tile([128, 8, DS], f32r, tag="w")
            wv_sb = wts.tile([128, 8, DS], f32r, tag="w")
            for et in range(8):
                nc.sync.dma_start(out=wq_sb[:, et, :], in_=wqT_d[et * 128 : et * 128 + 128, :])
                nc.scalar.dma_start(out=wk_sb[:, et, :], in_=wkT_d[et * 128 : et * 128 + 128, :])
                nc.gpsimd.dma_start(out=wv_sb[:, et, :], in_=wvT_d[et * 128 : et * 128 + 128, :])

            # ---- persistent activations ----
            qT = singles.tile([128, 2, 4, 512], f32r)  # [p, dt, nb, n]
            kT = singles.tile([128, 2, M], f32r)  # [p, dt, m]
            v_sb = singles.tile([128, n_mt, 4, C + 1], f32r)  # [m_p, mt, h, c|ones]
            xT = singles.tile([128, 2, 4, 512], f32r)  # [p, dt, nb, n]
            nc.vector.tensor_copy(
                v_sb[:, :, :, C], onesv.rearrange("p (a b) -> p a b", a=n_mt)
            )

            ccq_in = dram.tile([2048], f32)
            ccq_out = dram.tile([2048], f32)
            cck_in = dram.tile([M], f32)
            cck_out = dram.tile([M], f32)
            rsq_dram = dram.tile([2048], f32)

            with (
                tc.tile_pool(name="projps", bufs=2, space="PSUM") as projps,
                tc.tile_pool(name="vps", bufs=2, space="PSUM") as vps,
                tc.tile_pool(name="ssps", bufs=2, space="PSUM") as ssps,
            ):
                # ---- q projection: qT[dt, nb] = Wq'^T-slice @ q-block ----
                for nb in range(4):
                    blk = blkpool.tile([128, 8, 512], f32r, tag="blk")
                    for et in range(8):
                        nc.sync.dma_start(
                            out=blk[:, et, :],
                            in_=qT_d[et * 128 : et * 128 + 128, nb * 512 : nb * 512 + 512],
                        )
                    ss_ps = ssps.tile([1, 512], f32, tag="ss")
                    for dt in range(2):
                        ps = projps.tile([128, 512], f32, tag="proj")
                        for et in range(8):
                            nc.tensor.matmul(
                                ps,
                                wq_sb[:, et, dt * 128 : dt * 128 + 128],
                                blk[:, et, :],
                                start=(et == 0),
                                stop=(et == 7),
                            )
                        nc.vector.tensor_copy(qT[:, dt, nb, :], ps)
                        sq = sqpool.tile([128, 512], f32r, tag="sq")
                        nc.vector.tensor_mul(sq, qT[:, dt, nb, :], qT[:, dt, nb, :])
                        nc.tensor.matmul(
                            ss_ps,
                            ig2q_sb[:, dt : dt + 1],
                            sq,
                            start=(dt == 0),
                            stop=(dt == 1),
                            skip_group_check=True,
                        )
                    ss_sb = small.tile([1, 512], f32, tag="ss_sb")
                    nc.scalar.copy(ss_sb, ss_ps)
                    nc.sync.dma_start(
                        out=ccq_in[nb * 512 : nb * 512 + 512].rearrange(
                            "(a n) -> a n", a=1
                        ),
                        in_=ss_sb,
                    )

                # AR_q fires while the kv projection runs; AR_k pipelines
                # behind it on the CC queue.
                nc.gpsimd.collective_compute(
                    "AllReduce",
                    mybir.AluOpType.add,
                    replica_groups=[[0, 1, 2, 3], [4, 5, 6, 7]],
                    ins=[ccq_in.opt()],
                    outs=[ccq_out.opt()],
                )

                # ---- fused k+v projection over one kfT stream ----
                for moff, w in mblocks:
                    blk = blkpool.tile([128, 8, 512], f32r, tag="blk")
                    for et in range(8):
                        nc.scalar.dma_start(
                            out=blk[:, et, 0:w],
                            in_=kfT_d[et * 128 : et * 128 + 128, moff : moff + w],
                        )
                    ss_ps = ssps.tile([1, 512], f32, tag="ss")
                    for dt in range(2):
                        ps = projps.tile([128, 512], f32, tag="proj")
                        for et in range(8):
                            nc.tensor.matmul(
                                ps[:, 0:w],
                                wk_sb[:, et, dt * 128 : dt * 128 + 128],
                                blk[:, et, 0:w],
                                start=(et == 0),
                                stop=(et == 7),
                            )
                        nc.vector.tensor_copy(kT[:, dt, moff : moff + w], ps[:, 0:w])
                        sq = sqpool.tile([128, 512], f32r, tag="sq")
                        nc.vector.tensor_mul(
                            sq[:, 0:w],
                            kT[:, dt, moff : moff + w],
                            kT[:, dt, moff : moff + w],
                        )
                        nc.tensor.matmul(
                            ss_ps[:, 0:w],
                            ig2k_sb[:, dt : dt + 1],
                            sq[:, 0:w],
                            start=(dt == 0),
                            stop=(dt == 1),
                            skip_group_check=True,
                        )
                    ss_sb = small.tile([1, 512], f32, tag="ss_sb")
                    nc.scalar.copy(ss_sb[:, 0:w], ss_ps[:, 0:w])
                    nc.sync.dma_start(
                        out=cck_in[moff : moff + w].rearrange(
                            "(a n) -> a n", a=1
                        ),
                        in_=ss_sb[:, 0:w],
                    )
                    # v from the same resident block
                    for ct in range(w // 128):
                        mtg = moff // 128 + ct
                        psv = vps.tile([128, 256], f32, tag="v")
                        for et in range(8):
                            nc.tensor.matmul(
                                psv,
                                blk[:, et, ct * 128 : ct * 128 + 128],
                                wv_sb[:, et, :],
                                start=(et == 0),
                                stop=(et == 7),
                            )
                        nc.vector.tensor_copy(
                            v_sb[:, mtg, :, 0:C],
                            psv.rearrange("p (h c) -> p h c", c=C),
                        )

                nc.gpsimd.collective_compute(
                    "AllReduce",
                    mybir.AluOpType.add,
                    replica_groups=[[0, 1, 2, 3], [4, 5, 6, 7]],
                    ins=[cck_in.opt()],
                    outs=[cck_out.opt()],
                )

                # wo weight load (DMA only; hides under the AllReduce)
                wo_sb = wts.tile([128, 2, D], f32r, tag="w")
                for dc in range(2):
                    nc.sync.dma_start(
                        out=wo_sb[:, dc, :], in_=woT_d[dc * 128 : dc * 128 + 128, :]
                    )

                # ---- warm burst: dependency-free matmuls spanning the
                # AllReduce latency so the PE HAM clock-gate stays at 8/8
                # when attention starts (a PE-idle window here would drop it
                # to 4/8 and the attention stream would start cold) ----
                warm = projps.tile([128, 512], f32, tag="proj")
                for i in range(WARM_MM):
                    nc.tensor.matmul(
                        warm,
                        kT[:, 0, 0:128],
                        kT[:, 0, 512:1024],
                        start=(i == 0),
                        stop=(i == WARM_MM - 1),
                        skip_group_check=True,
                    )
                warm_sink = small.tile([1, 512], f32, tag="rd")
                nc.vector.tensor_copy(warm_sink, warm[0:1, :])

                # ---- rstd_k: [128, n_mt] lane-parallel; feeds exp scale ----
                ss128 = small.tile([128, n_mt], f32, tag="ssk")
                nc.sync.dma_start(
                    out=ss128, in_=cck_out.rearrange("(t p) -> p t", p=128)
                )
                stdk = small.tile([128, n_mt], f32, tag="stdk")
                nc.scalar.activation(stdk, ss128, AF.Sqrt, bias=eps_t, scale=invd_t)
                rstdk = singles.tile([128, n_mt], f32)
                nc.vector.reciprocal_approx_fast(out=rstdk, in_=stdk)

                # ---- rstd_q: lane-parallel [128, 16], then a DRAM bounce to
                # the [1, 2048] row layout the bcast outer-product wants
                # (single-lane sqrt/recip on [1, 2048] costs ~5us; this ~1us)
                ssq128 = small.tile([128, 16], f32, tag="ssq")
                nc.sync.dma_start(
                    out=ssq128, in_=ccq_out.rearrange("(t p) -> p t", p=128)
                )
                stdq = small.tile([128, 16], f32, tag="stdq")
                nc.scalar.activation(stdq, ssq128, AF.Sqrt, bias=eps_t, scale=invd_t)
                rsq128 = small.tile([128, 16], f32, tag="rsq")
                nc.vector.reciprocal_approx_fast(out=rsq128, in_=stdq)
                nc.sync.dma_start(
                    out=rsq_dram.rearrange("(t p) -> p t", p=128), in_=rsq128
                )
                rs_row = singles.tile([1, 2048], f32)
                nc.sync.dma_start(
                    out=rs_row, in_=rsq_dram.rearrange("(a n) -> a n", a=1)
                )
                # q finalize: qT[d, n] *= rstd_q[n] via ones outer-product bcast
                for nb in range(4):
                    bcq = projps.tile([128, 512], f32, tag="proj")
                    nc.tensor.matmul(
                        bcq,
                        ones1x128,
                        rs_row[:, nb * 512 : nb * 512 + 512],
                        start=True,
                        stop=True,
                    )
                    for dt in range(2):
                        nc.vector.tensor_mul(qT[:, dt, nb, :], qT[:, dt, nb, :], bcq)

            # ---- attention: flat stream over g = (pass, mt), pipelined
            # depth-2 ACROSS pass boundaries.  PE order:
            #   ... PV(g-1), QK(g+1), filler, PV(g), QK(g+2), ...
            # ACT order: exp(0), exp(1), ...  The PE never waits on an exp
            # (QK(g+1)'s s2 slot was freed by exp(g-1), one full exp ago) so
            # the HAM clock-gate stays warm.
            # PSUM banks: s2 2x2 + o2 2 + filler/bc/dummy 2x1 = 8.
            with (
                tc.tile_pool(name="sps", bufs=2, space="PSUM") as spool,
                tc.tile_pool(name="ops", bufs=1, space="PSUM") as opool,
                tc.tile_pool(name="fps", bufs=2, space="PSUM") as fpool,
            ):
                passes = [(nbp, h) for nbp in range(2) for h in range(4)]
                G = len(passes) * n_mt
                s2g, p2g, o2cur = {}, {}, {}
                filler = []
                prev = [None]

                def emit_dummy():
                    dum = fpool.tile([128, 512], f32, tag="f")
                    nc.tensor.matmul(
                        dum, kT[:, 0, 0:128], kT[:, 0, 512:1024],
                        start=True, stop=True, skip_group_check=True,
                    )

                def make_outproj(nbp):
                    """one closure per (nb, ot) psum tile: 2 matmuls + copy + dma"""
                    fns = []
                    for nb in (2 * nbp, 2 * nbp + 1):
                        for ot in range(8):
                            def f(nb=nb, ot=ot):
                                ps = fpool.tile([128, 512], f32, tag="f")
                                for dc in range(2):
                                    nc.tensor.matmul(
                                        ps,
                                        wo_sb[:, dc, ot * 128 : ot * 128 + 128],
                                        xT[:, dc, nb, :],
                                        start=(dc == 0),
                                        stop=(dc == 1),
                                    )
                                out_sb = outbuf.tile([128, 512], f32, tag="osb")
                                nc.vector.tensor_copy(out_sb, ps)
                                nc.sync.dma_start(
                                    out=outT_d[
                                        ot * 128 : ot * 128 + 128,
                                        nb * 512 : nb * 512 + 512,
                                    ],
                                    in_=out_sb,
                                )
                            fns.append(f)
                    return fns

                def emit_normalize(state):
                    """bc outer-products + muls for a pass whose DVE recips are
                    done by now (emitted one pass late to keep PE gapless)."""
                    hh, nbp_, oo_sb, rds_ = state
                    ddt, ooff = hh // 2, (hh % 2) * C
                    for i, nb in enumerate((2 * nbp_, 2 * nbp_ + 1)):
                        bc = fpool.tile([128, 512], f32, tag="f")
                        nc.tensor.matmul(
                            bc[0:C, :], ones1x64, rds_[i], start=True, stop=True
                        )
                        nc.vector.tensor_mul(
                            xT[ooff : ooff + C, ddt, nb, :],
                            oo_sb[0:C, i, :],
                            bc[0:C, :],
                        )

                def emit_qk(g):
                    (nbp, h), mt = passes[g // n_mt], g % n_mt
                    dt, off = h // 2, (h % 2) * C
                    s2 = spool.tile([128, 2, 512], f32, tag="s2")
                    kT_lhs = kT[off : off + C, dt, mt * 128 : mt * 128 + 128]
                    for i, nb in enumerate((2 * nbp, 2 * nbp + 1)):
                        nc.tensor.matmul(
                            s2[:, i, :],
                            kT_lhs,
                            qT[off : off + C, dt, nb, :],
                            start=True,
                            stop=True,
                        )
                    s2g[g] = s2

                def emit_exp(g):
                    mt = g % n_mt
                    p2 = ppool.tile([128, 2, 512], f32r, tag="p")
                    nc.scalar.activation(
                        p2, s2g.pop(g), AF.Exp,
                        bias=mb_sb[:, mt : mt + 1],
                        scale=rstdk[:, mt : mt + 1],
                    )
                    p2g[g] = p2

                def emit_pv(g):
                    pi, mt = g // n_mt, g % n_mt
                    nbp, h = passes[pi]
                    if mt == 0:
                        o2_t = opool.tile([C + 1, 2, 512], f32, tag="o2")
                        o2cur[pi] = o2_t
                    o2 = o2cur[pi]
                    p2 = p2g.pop(g)
                    for i in range(2):
                        nc.tensor.matmul(
                            o2[:, i, :],
                            v_sb[:, mt, h, :],
                            p2[:, i, :],
                            start=(mt == 0),
                            stop=(mt == n_mt - 1),
                            skip_group_check=True,
                        )
                    if mt == n_mt - 1:
                        # pass drain: free o2 with one DVE copy; recips; the
                        # PREVIOUS pass's normalize (its recips are done)
                        o_sb = obuf.tile([C + 1, 2, 512], f32, tag="osb")
                        nc.vector.tensor_copy(o_sb, o2cur.pop(pi))
                        rds = []
                        for i in range(2):
                            den = rdp.tile([1, 512], f32, tag="den")
                            nc.vector.tensor_copy(den, o_sb[C : C + 1, i, :])
                            rd = rdp.tile([1, 512], f32, tag="rd")
                            nc.vector.reciprocal_approx_fast(out=rd, in_=den)
                            rds.append(rd)
                        if prev[0] is not None:
                            emit_normalize(prev[0])
                            if prev[0][1] == 0 and prev[0][0] == 3:
                                # nbp=0 fully normalized -> its out-projection
                                # becomes PE filler for the remaining passes
                                filler.extend(make_outproj(0))
                        prev[0] = (h, nbp, o_sb, rds)

                emit_qk(0)
                emit_qk(1)
                emit_exp(0)
                for g in range(1, G):
                    emit_pv(g - 1)
                    if g + 1 < G:
                        emit_qk(g + 1)
                    if filler:
                        filler.pop(0)()
                    else:
                        emit_dummy()
                    emit_exp(g)
                emit_pv(G - 1)

                emit_normalize(prev[0])
                # tail: whatever filler wasn't consumed + second-half out proj
                for f in filler:
                    f()
                for f in make_outproj(1):
                    f()

    nc.finalize()
    return nc


_NC_CACHE = {}


def _get_nc(n_mt: int):
    if n_mt not in _NC_CACHE:
        _NC_CACHE[n_mt] = build(n_mt)
    return _NC_CACHE[n_mt]


def n_mt_for(mask) -> int:
    mask = np.asarray(mask)
    cnt = int(max((mask[b] != 0).sum() for b in range(B)))
    return max(1, (cnt + 127) // 128)


def make_in_maps(querys, key_feats, mask, Wq, Wk, Wv, gq, gk, Wo, bo, n_mt):
    M = n_mt * 128
    querys = np.asarray(querys, dtype=np.float32)
    key_feats = np.asarray(key_feats, dtype=np.float32)
    mask = np.asarray(mask)
    gq = np.asarray(gq, dtype=np.float32)
    gk = np.asarray(gk, dtype=np.float32)

    gsq_full = gq * np.float32(SCALE)  # folded into Wq rows
    gsk_full = gk.astype(np.float32)  # folded into Wk rows
    Wq_f = np.asarray(Wq, dtype=np.float32) * gsq_full[:, None]
    Wk_f = np.asarray(Wk, dtype=np.float32) * gsk_full[:, None]

    qT = [round_f32r(querys[b].T) for b in range(B)]
    kfT, mb = [], []
    for b in range(B):
        idx = np.flatnonzero(mask[b] != 0)
        cnt = len(idx)
        kc = np.zeros((M, E), np.float32)
        kc[:cnt] = key_feats[b][idx]
        kfT.append(round_f32r(kc.T))
        mbv = np.full((M,), NEG, dtype=np.float32)
        mbv[:cnt] = 0.0
        mb.append(mbv.reshape(n_mt, 128))

    wqT, wkT, wvT, woT, ig2q, ig2k = [], [], [], [], [], []
    for j in range(4):
        dsl = slice(j * DS, (j + 1) * DS)
        wqT.append(round_f32r(Wq_f[dsl].T))
        wkT.append(round_f32r(Wk_f[dsl].T))
        wvT.append(round_f32r(np.asarray(Wv)[dsl].T))
        woT.append(round_f32r(np.asarray(Wo)[:, dsl].T))
        # sumsq compensation: raw sumsq = sum_d (q'_d)^2 / gs_d^2
        ig2q.append(round_f32r((1.0 / gsq_full[dsl] ** 2).reshape(2, 128)))
        ig2k.append(round_f32r((1.0 / gsk_full[dsl] ** 2).reshape(2, 128)))

    in_maps = []
    for cid in range(NCORES):
        b, j = cid // 4, cid % 4
        in_maps.append(
            {
                "qT": qT[b],
                "kfT": kfT[b],
                "wqT": wqT[j],
                "wkT": wkT[j],
                "wvT": wvT[j],
                "woT": woT[j],
                "ig2q": ig2q[j],
                "ig2k": ig2k[j],
                "mbias": mb[b],
            }
        )
    return in_maps


def assemble(results, bo):
    bo = np.asarray(bo, dtype=np.float32)
    out = np.zeros((B, N, D), dtype=np.float32)
    for cid in range(NCORES):
        b = cid // 4
        out[b] += results[cid]["outT"].T
    out += bo
    return out


def kernel(querys, key_feats, mask, Wq, Wk, Wv, gq, gk, Wo, bo):
    n_mt = n_mt_for(mask)
    nc = _get_nc(n_mt)
    in_maps = make_in_maps(querys, key_feats, mask, Wq, Wk, Wv, gq, gk, Wo, bo, n_mt)
    res = run_bass_kernel_spmd(nc, in_maps, list(range(NCORES)))
    return assemble(res.results, bo)


# revision 14
# speedup vs baseline: 1.0057x; 1.0057x over previous
"""CrossAttention Trainium2 kernel (8 NeuronCores).

Reference computation (B=2, N=M=2048, D=1024, H=16, C=64):
    q = rmsnorm(querys @ Wq.T, gq) * C**-0.5       [B,N,D]
    k = rmsnorm(key_feats @ Wk.T, gk)              [B,M,D]
    v = key_feats @ Wv.T                           [B,M,D]
    attn = softmax(mask(q @ k.T per head))         [B,H,N,M]
    out = (attn @ v per head, concat) @ Wo.T + bo  [B,N,D]

Sharding: core = b*4 + j (b in {0,1}; j in {0..3} owns heads 4j..4j+3 = a
256-wide slice of D). Host pre-transposes inputs/weights, folds gq*scale /
gk into Wq / Wk rows, and pre-rounds everything to f32r (fp32 with 11-bit
mantissa -> full PE rate).

v2 structural changes vs v1:
  - Mask compaction: rows with mask==0 contribute exp(-inf)=0 to both the
    softmax denominator and PV, so the host gathers only the valid kf
    columns (per batch), pads to a multiple of 128, and the kernel runs
    with M_pad ~= 1152 instead of 2048.  All M-side work (k/v projection,
    QK, exp, PV, kf DMA) shrinks ~2x.  Padding columns carry bias -1e30
    into the exp -> contribute exactly 0.
  - k and v projections fused over a single kfT stream (halves kf DMA).
  - One fused AllReduce carries both q and k partial sum-of-squares
    (2048 + M_pad floats); its ~27us mesh latency is bridged by a long
    dependency-free dummy-matmul burst that keeps the PE HAM clock-gate
    warm (K=8/8) into attention.
  - Attention is software-pipelined depth-2 per (nbp, h) pass: PE order is
    ... PV(mt-1), QK(mt+1), filler, PV(mt) ... so the PE never stalls on
    the ACT exp (v1 stalled ~0.4us every mt, which kept HAM at K=4/8 =
    1.2 GHz for the whole 314us attention phase).  ACT exp (~1.2us/mt) is
    the pace-setter; PE real work is ~1.0us/mt, padded by a dummy matmul
    (first n-half) or an out-projection matmul pair (second n-half).
  - The out projection (partial over this core's d-slice) is interleaved
    into attention as filler work; the host sums 4 partials per b and
    adds bo.
"""

import numpy as np

import concourse.tile as tile
from concourse import bacc, mybir
from concourse.bass_utils import run_bass_kernel_spmd

B, N, M_FULL, D, H = 2, 2048, 2048, 1024, 16
C = D // H  # 64, head dim
E = D  # input feature dim
EPS = 1e-6
SCALE = C ** (-0.5)
DS = D // 4  # 256, per-core d-slice
NCORES = 8

f32 = mybir.dt.float32
f32r = mybir.dt.float32r
AF = mybir.ActivationFunctionType

NEG = -1e30
WARM_MM = 90  # dummy matmuls bridging the AllReduce latency into attention


def round_f32r(x: np.ndarray) -> np.ndarray:
    b = np.ascontiguousarray(x, dtype=np.float32).view(np.uint32)
    b = (b + 0x800) & np.uint32(0xFFFFF000)
    return b.view(np.float32)


def build(n_mt: int):
    M = n_mt * 128
    mblocks = []
    off = 0
    while off < M:
        w = min(512, M - off)
        mblocks.append((off, w))
        off += w

    nc = bacc.Bacc(None, target_bir_lowering=False)

    qT_d = nc.declare_dram_parameter("qT", [E, N], f32r, isOutput=False)
    kfT_d = nc.declare_dram_parameter("kfT", [E, M], f32r, isOutput=False)
    wqT_d = nc.declare_dram_parameter("wqT", [E, DS], f32r, isOutput=False)
    wkT_d = nc.declare_dram_parameter("wkT", [E, DS], f32r, isOutput=False)
    wvT_d = nc.declare_dram_parameter("wvT", [E, DS], f32r, isOutput=False)
    woT_d = nc.declare_dram_parameter("woT", [DS, D], f32r, isOutput=False)
    ig2q_d = nc.declare_dram_parameter("ig2q", [2, 128], f32r, isOutput=False)
    ig2k_d = nc.declare_dram_parameter("ig2k", [2, 128], f32r, isOutput=False)
    mb_d = nc.declare_dram_parameter("mbias", [n_mt, 128], f32, isOutput=False)
    outT_d = nc.declare_dram_parameter("outT", [D, N], f32, isOutput=True)

    with (
        nc.allow_low_precision(reason="f32r matmul operands by design; fp32 PSUM"),
        tile.TileContext(nc) as tc,
    ):
        with (
            tc.tile_pool(name="singles", bufs=1) as singles,
            tc.tile_pool(name="wts", bufs=3) as wts,
            tc.tile_pool(name="blk", bufs=2) as blkpool,
            tc.tile_pool(name="sq", bufs=2) as sqpool,
            tc.tile_pool(name="psb", bufs=3) as ppool,
            tc.tile_pool(name="obuf", bufs=2) as obuf,
            tc.tile_pool(name="osb2", bufs=2) as outbuf,
            tc.tile_pool(name="rdp", bufs=6) as rdp,
            tc.tile_pool(name="small", bufs=2) as small,
            tc.tile_pool(name="dram", bufs=1, space="DRAM") as dram,
        ):
            # ---- constants / small inputs ----
            ones1x64 = singles.tile([1, 64], f32)
            nc.vector.memset(ones1x64, 1.0)
            ones1x128 = singles.tile([1, 128], f32)
            nc.vector.memset(ones1x128, 1.0)
            onesv = singles.tile([128, n_mt * 4], f32)
            nc.vector.memset(onesv, 1.0)
            eps_t = singles.tile([128, 1], f32)
            nc.vector.memset(eps_t, EPS)
            invd_t = singles.tile([128, 1], f32)
            nc.vector.memset(invd_t, 1.0 / D)
            ig2q_sb = singles.tile([128, 2], f32r)
            nc.sync.dma_start(out=ig2q_sb, in_=ig2q_d.rearrange("t p -> p t"))
            ig2k_sb = singles.tile([128, 2], f32r)
            nc.sync.dma_start(out=ig2k_sb, in_=ig2k_d.rearrange("t p -> p t"))
            mb_sb = singles.tile([128, n_mt], f32)
            nc.sync.dma_start(out=mb_sb, in_=mb_d.rearrange("t p -> p t"))

            # weights: wq, wk, wv upfront; wo reuses wq's slot after q proj
            wq_sb = wts.tile([128, 8, DS], f32r, tag="w")
            wk_sb = wts.tile([128, 8, DS], f32r, tag="w")
            wv_sb = wts.tile([128, 8, DS], f32r, tag="w")
            for et in range(8):
                nc.sync.dma_start(out=wq_sb[:, et, :], in_=wqT_d[et * 128 : et * 128 + 128, :])
                nc.scalar.dma_start(out=wk_sb[:, et, :], in_=wkT_d[et * 128 : et * 128 + 128, :])
                nc.scalar.dma_start(out=wv_sb[:, et, :], in_=wvT_d[et * 128 : et * 128 + 128, :])

            # ---- persistent activations ----
            qT = singles.tile([128, 2, 4, 512], f32r)  # [p, dt, nb, n]
            kT = singles.tile([128, 2, M], f32r)  # [p, dt, m]
            v_sb = singles.tile([128, n_mt, 4, C + 1], f32r)  # [m_p, mt, h, c|ones]
            xT = singles.tile([128, 2, 4, 512], f32r)  # [p, dt, nb, n]
            nc.vector.tensor_copy(
                v_sb[:, :, :, C], onesv.rearrange("p (a b) -> p a b", a=n_mt)
            )

            ccq_in = dram.tile([2048], f32)
            ccq_out = dram.tile([2048], f32)
            cck_in = dram.tile([M], f32)
            cck_out = dram.tile([M], f32)
            rsq_dram = dram.tile([2048], f32)

            with (
                tc.tile_pool(name="projps", bufs=2, space="PSUM") as projps,
                tc.tile_pool(name="vps", bufs=2, space="PSUM") as vps,
                tc.tile_pool(name="ssps", bufs=2, space="PSUM") as ssps,
            ):
                # ---- fused k+v projection over one kfT stream ----
                for moff, w in mblocks:
                    blk = blkpool.tile([128, 8, 512], f32r, tag="blk")
                    for et in range(8):
                        nc.scalar.dma_start(
                            out=blk[:, et, 0:w],
                            in_=kfT_d[et * 128 : et * 128 + 128, moff : moff + w],
                        )
                    ss_ps = ssps.tile([1, 512], f32, tag="ss")
                    for dt in range(2):
                        ps = projps.tile([128, 512], f32, tag="proj")
                        for et in range(8):
                            nc.tensor.matmul(
                                ps[:, 0:w],
                                wk_sb[:, et, dt * 128 : dt * 128 + 128],
                                blk[:, et, 0:w],
                                start=(et == 0),
                                stop=(et == 7),
                            )
                        nc.vector.tensor_copy(kT[:, dt, moff : moff + w], ps[:, 0:w])
                        sq = sqpool.tile([128, 512], f32r, tag="sq")
                        nc.vector.tensor_mul(
                            sq[:, 0:w],
                            kT[:, dt, moff : moff + w],
                            kT[:, dt, moff : moff + w],
                        )
                        nc.tensor.matmul(
                            ss_ps[:, 0:w],
                            ig2k_sb[:, dt : dt + 1],
                            sq[:, 0:w],
                            start=(dt == 0),
                            stop=(dt == 1),
                            skip_group_check=True,
                        )
                    ss_sb = small.tile([1, 512], f32, tag="ss_sb")
                    nc.scalar.copy(ss_sb[:, 0:w], ss_ps[:, 0:w])
                    nc.sync.dma_start(
                        out=cck_in[moff : moff + w].rearrange(
                            "(a n) -> a n", a=1
                        ),
                        in_=ss_sb[:, 0:w],
                    )
                    # v from the same resident block
                    for ct in range(w // 128):
                        mtg = moff // 128 + ct
                        psv = vps.tile([128, 256], f32, tag="v")
                        for et in range(8):
                            nc.tensor.matmul(
                                psv,
                                blk[:, et, ct * 128 : ct * 128 + 128],
                                wv_sb[:, et, :],
                                start=(et == 0),
                                stop=(et == 7),
                            )
                        nc.vector.tensor_copy(
                            v_sb[:, mtg, :, 0:C],
                            psv.rearrange("p (h c) -> p h c", c=C),
                        )

                nc.gpsimd.collective_compute(
                    "AllReduce",
                    mybir.AluOpType.add,
                    replica_groups=[[0, 1, 2, 3], [4, 5, 6, 7]],
                    ins=[cck_in.opt()],
                    outs=[cck_out.opt()],
                )

                # ---- q projection: qT[dt, nb] = Wq'^T-slice @ q-block ----
                for nb in range(4):
                    blk = blkpool.tile([128, 8, 512], f32r, tag="blk")
                    for et in range(8):
                        nc.sync.dma_start(
                            out=blk[:, et, :],
                            in_=qT_d[et * 128 : et * 128 + 128, nb * 512 : nb * 512 + 512],
                        )
                    ss_ps = ssps.tile([1, 512], f32, tag="ss")
                    for dt in range(2):
                        ps = projps.tile([128, 512], f32, tag="proj")
                        for et in range(8):
                            nc.tensor.matmul(
                                ps,
                                wq_sb[:, et, dt * 128 : dt * 128 + 128],
                                blk[:, et, :],
                                start=(et == 0),
                                stop=(et == 7),
                            )
                        nc.vector.tensor_copy(qT[:, dt, nb, :], ps)
                        sq = sqpool.tile([128, 512], f32r, tag="sq")
                        nc.vector.tensor_mul(sq, qT[:, dt, nb, :], qT[:, dt, nb, :])
                        nc.tensor.matmul(
                            ss_ps,
                            ig2q_sb[:, dt : dt + 1],
                            sq,
                            start=(dt == 0),
                            stop=(dt == 1),
                            skip_group_check=True,
                        )
                    ss_sb = small.tile([1, 512], f32, tag="ss_sb")
                    nc.scalar.copy(ss_sb, ss_ps)
                    nc.sync.dma_start(
                        out=ccq_in[nb * 512 : nb * 512 + 512].rearrange(
                            "(a n) -> a n", a=1
                        ),
                        in_=ss_sb,
                    )

                # AR_q fires while the kv projection runs; AR_k pipelines
                # behind it on the CC queue.
                nc.gpsimd.collective_compute(
                    "AllReduce",
                    mybir.AluOpType.add,
                    replica_groups=[[0, 1, 2, 3], [4, 5, 6, 7]],
                    ins=[ccq_in.opt()],
                    outs=[ccq_out.opt()],
                )

                # wo weight load (DMA only; hides under the AllReduce)
                wo_sb = wts.tile([128, 2, D], f32r, tag="w")
                for dc in range(2):
                    nc.sync.dma_start(
                        out=wo_sb[:, dc, :], in_=woT_d[dc * 128 : dc * 128 + 128, :]
                    )

                # ---- warm burst: dependency-free matmuls spanning the
                # AllReduce latency so the PE HAM clock-gate stays at 8/8
                # when attention starts (a PE-idle window here would drop it
                # to 4/8 and the attention stream would start cold) ----
                warm = projps.tile([128, 512], f32, tag="proj")
                for i in range(WARM_MM):
                    nc.tensor.matmul(
                        warm,
                        kT[:, 0, 0:128],
                        kT[:, 0, 512:1024],
                        start=(i == 0),
                        stop=(i == WARM_MM - 1),
                        skip_group_check=True,
                    )
                warm_sink = small.tile([1, 512], f32, tag="rd")
                nc.vector.tensor_copy(warm_sink, warm[0:1, :])

                # ---- rstd_k: [128, n_mt] lane-parallel; feeds exp scale ----
                ss128 = small.tile([128, n_mt], f32, tag="ssk")
                nc.sync.dma_start(
                    out=ss128, in_=cck_out.rearrange("(t p) -> p t", p=128)
                )
                stdk = small.tile([128, n_mt], f32, tag="stdk")
                nc.scalar.activation(stdk, ss128, AF.Sqrt, bias=eps_t, scale=invd_t)
                rstdk = singles.tile([128, n_mt], f32)
                nc.vector.reciprocal_approx_fast(out=rstdk, in_=stdk)

                # ---- rstd_q: lane-parallel [128, 16], then a DRAM bounce to
                # the [1, 2048] row layout the bcast outer-product wants
                # (single-lane sqrt/recip on [1, 2048] costs ~5us; this ~1us)
                ssq128 = small.tile([128, 16], f32, tag="ssq")
                nc.sync.dma_start(
                    out=ssq128, in_=ccq_out.rearrange("(t p) -> p t", p=128)
                )
                stdq = small.tile([128, 16], f32, tag="stdq")
                nc.scalar.activation(stdq, ssq128, AF.Sqrt, bias=eps_t, scale=invd_t)
                rsq128 = small.tile([128, 16], f32, tag="rsq")
                nc.vector.reciprocal_approx_fast(out=rsq128, in_=stdq)
                nc.sync.dma_start(
                    out=rsq_dram.rearrange("(t p) -> p t", p=128), in_=rsq128
                )
                rs_row = singles.tile([1, 2048], f32)
                nc.sync.dma_start(
                    out=rs_row, in_=rsq_dram.rearrange("(a n) -> a n", a=1)
                )
                # q finalize: qT[d, n] *= rstd_q[n] via ones outer-product bcast
                for nb in range(4):
                    bcq = projps.tile([128, 512], f32, tag="proj")
                    nc.tensor.matmul(
                        bcq,
                        ones1x128,
                        rs_row[:, nb * 512 : nb * 512 + 512],
                        start=True,
                        stop=True,
                    )
                    for dt in range(2):
                        nc.vector.tensor_mul(qT[:, dt, nb, :], qT[:, dt, nb, :], bcq)

            # ---- attention: flat stream over g = (pass, mt), pipelined
            # depth-2 ACROSS pass boundaries.  PE order:
            #   ... PV(g-1), QK(g+1), filler, PV(g), QK(g+2), ...
            # ACT order: exp(0), exp(1), ...  The PE never waits on an exp
            # (QK(g+1)'s s2 slot was freed by exp(g-1), one full exp ago) so
            # the HAM clock-gate stays warm.
            # PSUM banks: s2 2x2 + o2 2 + filler/bc/dummy 2x1 = 8.
            with (
                tc.tile_pool(name="sps", bufs=2, space="PSUM") as spool,
                tc.tile_pool(name="ops", bufs=1, space="PSUM") as opool,
                tc.tile_pool(name="fps", bufs=2, space="PSUM") as fpool,
            ):
                passes = [(nbp, h) for nbp in range(2) for h in range(4)]
                G = len(passes) * n_mt
                s2g, p2g, o2cur = {}, {}, {}
                filler = []
                prev = [None]

                def emit_dummy():
                    dum = fpool.tile([128, 512], f32, tag="f")
                    nc.tensor.matmul(
                        dum, kT[:, 0, 0:128], kT[:, 0, 512:1024],
                        start=True, stop=True, skip_group_check=True,
                    )

                def make_outproj(nbp):
                    """one closure per (nb, ot) psum tile: 2 matmuls + copy + dma"""
                    fns = []
                    for nb in (2 * nbp, 2 * nbp + 1):
                        for ot in range(8):
                            def f(nb=nb, ot=ot):
                                ps = fpool.tile([128, 512], f32, tag="f")
                                for dc in range(2):
                                    nc.tensor.matmul(
                                        ps,
                                        wo_sb[:, dc, ot * 128 : ot * 128 + 128],
                                        xT[:, dc, nb, :],
                                        start=(dc == 0),
                                        stop=(dc == 1),
                                    )
                                out_sb = outbuf.tile([128, 512], f32, tag="osb")
                                nc.vector.tensor_copy(out_sb, ps)
                                nc.sync.dma_start(
                                    out=outT_d[
                                        ot * 128 : ot * 128 + 128,
                                        nb * 512 : nb * 512 + 512,
                                    ],
                                    in_=out_sb,
                                )
                            fns.append(f)
                    return fns

                def emit_normalize(state):
                    """bc outer-products + muls for a pass whose DVE recips are
                    done by now (emitted one pass late to keep PE gapless)."""
                    hh, nbp_, oo_sb, rds_ = state
                    ddt, ooff = hh // 2, (hh % 2) * C
                    for i, nb in enumerate((2 * nbp_, 2 * nbp_ + 1)):
                        bc = fpool.tile([128, 512], f32, tag="f")
                        nc.tensor.matmul(
                            bc[0:C, :], ones1x64, rds_[i], start=True, stop=True
                        )
                        nc.vector.tensor_mul(
                            xT[ooff : ooff + C, ddt, nb, :],
                            oo_sb[0:C, i, :],
                            bc[0:C, :],
                        )

                def emit_qk(g):
                    (nbp, h), mt = passes[g // n_mt], g % n_mt
                    dt, off = h // 2, (h % 2) * C
                    s2 = spool.tile([128, 2, 512], f32, tag="s2")
                    kT_lhs = kT[off : off + C, dt, mt * 128 : mt * 128 + 128]
                    for i, nb in enumerate((2 * nbp, 2 * nbp + 1)):
                        nc.tensor.matmul(
                            s2[:, i, :],
                            kT_lhs,
                            qT[off : off + C, dt, nb, :],
                            start=True,
                            stop=True,
                        )
                    s2g[g] = s2

                def emit_exp(g):
                    mt = g % n_mt
                    p2 = ppool.tile([128, 2, 512], f32r, tag="p")
                    nc.scalar.activation(
                        p2, s2g.pop(g), AF.Exp,
                        bias=mb_sb[:, mt : mt + 1],
                        scale=rstdk[:, mt : mt + 1],
                    )
                    p2g[g] = p2

                def emit_pv(g):
                    pi, mt = g // n_mt, g % n_mt
                    nbp, h = passes[pi]
                    if mt == 0:
                        o2_t = opool.tile([C + 1, 2, 512], f32, tag="o2")
                        o2cur[pi] = o2_t
                    o2 = o2cur[pi]
                    p2 = p2g.pop(g)
                    for i in range(2):
                        nc.tensor.matmul(
                            o2[:, i, :],
                            v_sb[:, mt, h, :],
                            p2[:, i, :],
                            start=(mt == 0),
                            stop=(mt == n_mt - 1),
                            skip_group_check=True,
                        )
                    if mt == n_mt - 1:
                        # pass drain: free o2 with one DVE copy; recips; the
                        # PREVIOUS pass's normalize (its recips are done)
                        o_sb = obuf.tile([C + 1, 2, 512], f32, tag="osb")
                        nc.vector.tensor_copy(o_sb, o2cur.pop(pi))
                        rds = []
                        for i in range(2):
                            den = rdp.tile([1, 512], f32, tag="den")
                            nc.vector.tensor_copy(den, o_sb[C : C + 1, i, :])
                            rd = rdp.tile([1, 512], f32, tag="rd")
                            nc.vector.reciprocal_approx_fast(out=rd, in_=den)
                            rds.append(rd)
                        if prev[0] is not None:
                            emit_normalize(prev[0])
                            if prev[0][1] == 0 and prev[0][0] == 3:
                                # nbp=0 fully normalized -> its out-projection
                                # becomes PE filler for the remaining passes
                                filler.extend(make_outproj(0))
                        prev[0] = (h, nbp, o_sb, rds)

                emit_qk(0)
                emit_qk(1)
                emit_exp(0)
                for _ in range(3):
                    emit_dummy()
                for g in range(1, G):
                    emit_pv(g - 1)
                    if g + 1 < G:
                        emit_qk(g + 1)
                    if filler:
                        filler.pop(0)()
                    else:
                        emit_dummy()
                    emit_exp(g)
                emit_pv(G - 1)

                emit_normalize(prev[0])
                # tail: whatever filler wasn't consumed + second-half out proj
                for f in filler:
                    f()
                for f in make_outproj(1):
                    f()

    nc.finalize()
    return nc


_NC_CACHE = {}


def _get_nc(n_mt: int):
    if n_mt not in _NC_CACHE:
        _NC_CACHE[n_mt] = build(n_mt)
    return _NC_CACHE[n_mt]


def n_mt_for(mask) -> int:
    mask = np.asarray(mask)
    cnt = int(max((mask[b] != 0).sum() for b in range(B)))
    return max(1, (cnt + 127) // 128)


def make_in_maps(querys, key_feats, mask, Wq, Wk, Wv, gq, gk, Wo, bo, n_mt):
    M = n_mt * 128
    querys = np.asarray(querys, dtype=np.float32)
    key_feats = np.asarray(key_feats, dtype=np.float32)
    mask = np.asarray(mask)
    gq = np.asarray(gq, dtype=np.float32)
    gk = np.asarray(gk, dtype=np.float32)

    gsq_full = gq * np.float32(SCALE)  # folded into Wq rows
    gsk_full = gk.astype(np.float32)  # folded into Wk rows
    Wq_f = np.asarray(Wq, dtype=np.float32) * gsq_full[:, None]
    Wk_f = np.asarray(Wk, dtype=np.float32) * gsk_full[:, None]

    qT = [round_f32r(querys[b].T) for b in range(B)]
    kfT, mb = [], []
    for b in range(B):
        idx = np.flatnonzero(mask[b] != 0)
        cnt = len(idx)
        kc = np.zeros((M, E), np.float32)
        kc[:cnt] = key_feats[b][idx]
        kfT.append(round_f32r(kc.T))
        mbv = np.full((M,), NEG, dtype=np.float32)
        mbv[:cnt] = 0.0
        mb.append(mbv.reshape(n_mt, 128))

    wqT, wkT, wvT, woT, ig2q, ig2k = [], [], [], [], [], []
    for j in range(4):
        dsl = slice(j * DS, (j + 1) * DS)
        wqT.append(round_f32r(Wq_f[dsl].T))
        wkT.append(round_f32r(Wk_f[dsl].T))
        wvT.append(round_f32r(np.asarray(Wv)[dsl].T))
        woT.append(round_f32r(np.asarray(Wo)[:, dsl].T))
        # sumsq compensation: raw sumsq = sum_d (q'_d)^2 / gs_d^2
        ig2q.append(round_f32r((1.0 / gsq_full[dsl] ** 2).reshape(2, 128)))
        ig2k.append(round_f32r((1.0 / gsk_full[dsl] ** 2).reshape(2, 128)))

    in_maps = []
    for cid in range(NCORES):
        b, j = cid // 4, cid % 4
        in_maps.append(
            {
                "qT": qT[b],
                "kfT": kfT[b],
                "wqT": wqT[j],
                "wkT": wkT[j],
                "wvT": wvT[j],
                "woT": woT[j],
                "ig2q": ig2q[j],
                "ig2k": ig2k[j],
                "mbias": mb[b],
            }
        )
    return in_maps


def assemble(results, bo):
    bo = np.asarray(bo, dtype=np.float32)
    out = np.zeros((B, N, D), dtype=np.float32)
    for cid in range(NCORES):
        b = cid // 4
        out[b] += results[cid]["outT"].T
    out += bo
    return out


def kernel(querys, key_feats, mask, Wq, Wk, Wv, gq, gk, Wo, bo):
    n_mt = n_mt_for(mask)
    nc = _get_nc(n_mt)
    in_maps = make_in_maps(querys, key_feats, mask, Wq, Wk, Wv, gq, gk, Wo, bo, n_mt)
    res = run_bass_kernel_spmd(nc, in_maps, list(range(NCORES)))
    return assemble(res.results, bo)


# revision 15
# speedup vs baseline: 1.0077x; 1.0020x over previous
"""CrossAttention Trainium2 kernel (8 NeuronCores).

Reference computation (B=2, N=M=2048, D=1024, H=16, C=64):
    q = rmsnorm(querys @ Wq.T, gq) * C**-0.5       [B,N,D]
    k = rmsnorm(key_feats @ Wk.T, gk)              [B,M,D]
    v = key_feats @ Wv.T                           [B,M,D]
    attn = softmax(mask(q @ k.T per head))         [B,H,N,M]
    out = (attn @ v per head, concat) @ Wo.T + bo  [B,N,D]

Sharding: core = b*4 + j (b in {0,1}; j in {0..3} owns heads 4j..4j+3 = a
256-wide slice of D). Host pre-transposes inputs/weights, folds gq*scale /
gk into Wq / Wk rows, and pre-rounds everything to f32r (fp32 with 11-bit
mantissa -> full PE rate).

v2 structural changes vs v1:
  - Mask compaction: rows with mask==0 contribute exp(-inf)=0 to both the
    softmax denominator and PV, so the host gathers only the valid kf
    columns (per batch), pads to a multiple of 128, and the kernel runs
    with M_pad ~= 1152 instead of 2048.  All M-side work (k/v projection,
    QK, exp, PV, kf DMA) shrinks ~2x.  Padding columns carry bias -1e30
    into the exp -> contribute exactly 0.
  - k and v projections fused over a single kfT stream (halves kf DMA).
  - One fused AllReduce carries both q and k partial sum-of-squares
    (2048 + M_pad floats); its ~27us mesh latency is bridged by a long
    dependency-free dummy-matmul burst that keeps the PE HAM clock-gate
    warm (K=8/8) into attention.
  - Attention is software-pipelined depth-2 per (nbp, h) pass: PE order is
    ... PV(mt-1), QK(mt+1), filler, PV(mt) ... so the PE never stalls on
    the ACT exp (v1 stalled ~0.4us every mt, which kept HAM at K=4/8 =
    1.2 GHz for the whole 314us attention phase).  ACT exp (~1.2us/mt) is
    the pace-setter; PE real work is ~1.0us/mt, padded by a dummy matmul
    (first n-half) or an out-projection matmul pair (second n-half).
  - The out projection (partial over this core's d-slice) is interleaved
    into attention as filler work; the host sums 4 partials per b and
    adds bo.
"""

import numpy as np

import concourse.tile as tile
from concourse import bacc, mybir
from concourse.bass_utils import run_bass_kernel_spmd

B, N, M_FULL, D, H = 2, 2048, 2048, 1024, 16
C = D // H  # 64, head dim
E = D  # input feature dim
EPS = 1e-6
SCALE = C ** (-0.5)
DS = D // 4  # 256, per-core d-slice
NCORES = 8

f32 = mybir.dt.float32
f32r = mybir.dt.float32r
AF = mybir.ActivationFunctionType

NEG = -1e30
WARM_MM = 60  # dummy matmuls bridging the AllReduce latency into attention


def round_f32r(x: np.ndarray) -> np.ndarray:
    b = np.ascontiguousarray(x, dtype=np.float32).view(np.uint32)
    b = (b + 0x800) & np.uint32(0xFFFFF000)
    return b.view(np.float32)


def build(n_mt: int):
    M = n_mt * 128
    mblocks = []
    off = 0
    while off < M:
        w = min(512, M - off)
        mblocks.append((off, w))
        off += w

    nc = bacc.Bacc(None, target_bir_lowering=False)

    qT_d = nc.declare_dram_parameter("qT", [E, N], f32r, isOutput=False)
    kfT_d = nc.declare_dram_parameter("kfT", [E, M], f32r, isOutput=False)
    wqT_d = nc.declare_dram_parameter("wqT", [E, DS], f32r, isOutput=False)
    wkT_d = nc.declare_dram_parameter("wkT", [E, DS], f32r, isOutput=False)
    wvT_d = nc.declare_dram_parameter("wvT", [E, DS], f32r, isOutput=False)
    woT_d = nc.declare_dram_parameter("woT", [DS, D], f32r, isOutput=False)
    ig2q_d = nc.declare_dram_parameter("ig2q", [2, 128], f32r, isOutput=False)
    ig2k_d = nc.declare_dram_parameter("ig2k", [2, 128], f32r, isOutput=False)
    mb_d = nc.declare_dram_parameter("mbias", [n_mt, 128], f32, isOutput=False)
    outT_d = nc.declare_dram_parameter("outT", [D, N], f32, isOutput=True)

    with (
        nc.allow_low_precision(reason="f32r matmul operands by design; fp32 PSUM"),
        tile.TileContext(nc) as tc,
    ):
        with (
            tc.tile_pool(name="singles", bufs=1) as singles,
            tc.tile_pool(name="wts", bufs=3) as wts,
            tc.tile_pool(name="blk", bufs=2) as blkpool,
            tc.tile_pool(name="sq", bufs=2) as sqpool,
            tc.tile_pool(name="psb", bufs=3) as ppool,
            tc.tile_pool(name="obuf", bufs=2) as obuf,
            tc.tile_pool(name="osb2", bufs=10) as outbuf,
            tc.tile_pool(name="rdp", bufs=4) as rdp,
            tc.tile_pool(name="small", bufs=2) as small,
            tc.tile_pool(name="dram", bufs=1, space="DRAM") as dram,
        ):
            # ---- constants / small inputs ----
            ones1x64 = singles.tile([1, 64], f32)
            nc.vector.memset(ones1x64, 1.0)
            ones1x128 = singles.tile([1, 128], f32)
            nc.vector.memset(ones1x128, 1.0)
            onesv = singles.tile([128, n_mt * 4], f32)
            nc.vector.memset(onesv, 1.0)
            eps_t = singles.tile([128, 1], f32)
            nc.vector.memset(eps_t, EPS)
            invd_t = singles.tile([128, 1], f32)
            nc.vector.memset(invd_t, 1.0 / D)
            ig2q_sb = singles.tile([128, 2], f32r)
            nc.sync.dma_start(out=ig2q_sb, in_=ig2q_d.rearrange("t p -> p t"))
            ig2k_sb = singles.tile([128, 2], f32r)
            nc.sync.dma_start(out=ig2k_sb, in_=ig2k_d.rearrange("t p -> p t"))
            mb_sb = singles.tile([128, n_mt], f32)
            nc.sync.dma_start(out=mb_sb, in_=mb_d.rearrange("t p -> p t"))

            # weights: wq, wk, wv upfront; wo reuses wq's slot after q proj
            wq_sb = wts.tile([128, 8, DS], f32r, tag="w")
            wk_sb = wts.tile([128, 8, DS], f32r, tag="w")
            wv_sb = wts.tile([128, 8, DS], f32r, tag="w")
            for et in range(8):
                nc.sync.dma_start(out=wq_sb[:, et, :], in_=wqT_d[et * 128 : et * 128 + 128, :])
                nc.scalar.dma_start(out=wk_sb[:, et, :], in_=wkT_d[et * 128 : et * 128 + 128, :])
                nc.scalar.dma_start(out=wv_sb[:, et, :], in_=wvT_d[et * 128 : et * 128 + 128, :])

            # ---- persistent activations ----
            qT = singles.tile([128, 2, 4, 512], f32r)  # [p, dt, nb, n]
            kT = singles.tile([128, 2, M], f32r)  # [p, dt, m]
            v_sb = singles.tile([128, n_mt, 4, C + 1], f32r)  # [m_p, mt, h, c|ones]
            xT = singles.tile([128, 2, 4, 512], f32r)  # [p, dt, nb, n]
            nc.vector.tensor_copy(
                v_sb[:, :, :, C], onesv.rearrange("p (a b) -> p a b", a=n_mt)
            )

            ccq_in = dram.tile([2048], f32)
            ccq_out = dram.tile([2048], f32)
            cck_in = dram.tile([M], f32)
            cck_out = dram.tile([M], f32)
            rsq_dram = dram.tile([2048], f32)

            with (
                tc.tile_pool(name="projps", bufs=2, space="PSUM") as projps,
                tc.tile_pool(name="vps", bufs=2, space="PSUM") as vps,
                tc.tile_pool(name="ssps", bufs=2, space="PSUM") as ssps,
            ):
                # ---- fused k+v projection over one kfT stream ----
                for moff, w in mblocks:
                    blk = blkpool.tile([128, 8, 512], f32r, tag="blk")
                    for et in range(8):
                        nc.scalar.dma_start(
                            out=blk[:, et, 0:w],
                            in_=kfT_d[et * 128 : et * 128 + 128, moff : moff + w],
                        )
                    ss_ps = ssps.tile([1, 512], f32, tag="ss")
                    for dt in range(2):
                        ps = projps.tile([128, 512], f32, tag="proj")
                        for et in range(8):
                            nc.tensor.matmul(
                                ps[:, 0:w],
                                wk_sb[:, et, dt * 128 : dt * 128 + 128],
                                blk[:, et, 0:w],
                                start=(et == 0),
                                stop=(et == 7),
                            )
                        nc.vector.tensor_copy(kT[:, dt, moff : moff + w], ps[:, 0:w])
                        sq = sqpool.tile([128, 512], f32r, tag="sq")
                        nc.vector.tensor_mul(
                            sq[:, 0:w],
                            kT[:, dt, moff : moff + w],
                            kT[:, dt, moff : moff + w],
                        )
                        nc.tensor.matmul(
                            ss_ps[:, 0:w],
                            ig2k_sb[:, dt : dt + 1],
                            sq[:, 0:w],
                            start=(dt == 0),
                            stop=(dt == 1),
                            skip_group_check=True,
                        )
                    ss_sb = small.tile([1, 512], f32, tag="ss_sb")
                    nc.scalar.copy(ss_sb[:, 0:w], ss_ps[:, 0:w])
                    nc.sync.dma_start(
                        out=cck_in[moff : moff + w].rearrange(
                            "(a n) -> a n", a=1
                        ),
                        in_=ss_sb[:, 0:w],
                    )
                    # v from the same resident block
                    for ct in range(w // 128):
                        mtg = moff // 128 + ct
                        psv = vps.tile([128, 256], f32, tag="v")
                        for et in range(8):
                            nc.tensor.matmul(
                                psv,
                                blk[:, et, ct * 128 : ct * 128 + 128],
                                wv_sb[:, et, :],
                                start=(et == 0),
                                stop=(et == 7),
                            )
                        nc.vector.tensor_copy(
                            v_sb[:, mtg, :, 0:C],
                            psv.rearrange("p (h c) -> p h c", c=C),
                        )

                nc.gpsimd.collective_compute(
                    "AllReduce",
                    mybir.AluOpType.add,
                    replica_groups=[[0, 1, 2, 3], [4, 5, 6, 7]],
                    ins=[cck_in.opt()],
                    outs=[cck_out.opt()],
                )

                # ---- q projection: qT[dt, nb] = Wq'^T-slice @ q-block ----
                for nb in range(4):
                    blk = blkpool.tile([128, 8, 512], f32r, tag="blk")
                    for et in range(8):
                        nc.sync.dma_start(
                            out=blk[:, et, :],
                            in_=qT_d[et * 128 : et * 128 + 128, nb * 512 : nb * 512 + 512],
                        )
                    ss_ps = ssps.tile([1, 512], f32, tag="ss")
                    for dt in range(2):
                        ps = projps.tile([128, 512], f32, tag="proj")
                        for et in range(8):
                            nc.tensor.matmul(
                                ps,
                                wq_sb[:, et, dt * 128 : dt * 128 + 128],
                                blk[:, et, :],
                                start=(et == 0),
                                stop=(et == 7),
                            )
                        nc.vector.tensor_copy(qT[:, dt, nb, :], ps)
                        sq = sqpool.tile([128, 512], f32r, tag="sq")
                        nc.vector.tensor_mul(sq, qT[:, dt, nb, :], qT[:, dt, nb, :])
                        nc.tensor.matmul(
                            ss_ps,
                            ig2q_sb[:, dt : dt + 1],
                            sq,
                            start=(dt == 0),
                            stop=(dt == 1),
                            skip_group_check=True,
                        )
                    ss_sb = small.tile([1, 512], f32, tag="ss_sb")
                    nc.scalar.copy(ss_sb, ss_ps)
                    nc.sync.dma_start(
                        out=ccq_in[nb * 512 : nb * 512 + 512].rearrange(
                            "(a n) -> a n", a=1
                        ),
                        in_=ss_sb,
                    )

                # AR_q fires while the kv projection runs; AR_k pipelines
                # behind it on the CC queue.
                nc.gpsimd.collective_compute(
                    "AllReduce",
                    mybir.AluOpType.add,
                    replica_groups=[[0, 1, 2, 3], [4, 5, 6, 7]],
                    ins=[ccq_in.opt()],
                    outs=[ccq_out.opt()],
                )

                # wo weight load (DMA only; hides under the AllReduce)
                wo_sb = wts.tile([128, 2, D], f32r, tag="w")
                for dc in range(2):
                    nc.sync.dma_start(
                        out=wo_sb[:, dc, :], in_=woT_d[dc * 128 : dc * 128 + 128, :]
                    )

                # ---- warm burst: dependency-free matmuls spanning the
                # AllReduce latency so the PE HAM clock-gate stays at 8/8
                # when attention starts (a PE-idle window here would drop it
                # to 4/8 and the attention stream would start cold) ----
                warm = projps.tile([128, 512], f32, tag="proj")
                for i in range(WARM_MM):
                    nc.tensor.matmul(
                        warm,
                        kT[:, 0, 0:128],
                        kT[:, 0, 512:1024],
                        start=(i == 0),
                        stop=(i == WARM_MM - 1),
                        skip_group_check=True,
                    )
                warm_sink = small.tile([1, 512], f32, tag="rd")
                nc.vector.tensor_copy(warm_sink, warm[0:1, :])

                # ---- rstd_k: [128, n_mt] lane-parallel; feeds exp scale ----
                ss128 = small.tile([128, n_mt], f32, tag="ssk")
                nc.sync.dma_start(
                    out=ss128, in_=cck_out.rearrange("(t p) -> p t", p=128)
                )
                stdk = small.tile([128, n_mt], f32, tag="stdk")
                nc.scalar.activation(stdk, ss128, AF.Sqrt, bias=eps_t, scale=invd_t)
                rstdk = singles.tile([128, n_mt], f32)
                nc.vector.reciprocal_approx_fast(out=rstdk, in_=stdk)

                # ---- rstd_q: lane-parallel [128, 16], then a DRAM bounce to
                # the [1, 2048] row layout the bcast outer-product wants
                # (single-lane sqrt/recip on [1, 2048] costs ~5us; this ~1us)
                ssq128 = small.tile([128, 16], f32, tag="ssq")
                nc.sync.dma_start(
                    out=ssq128, in_=ccq_out.rearrange("(t p) -> p t", p=128)
                )
                stdq = small.tile([128, 16], f32, tag="stdq")
                nc.scalar.activation(stdq, ssq128, AF.Sqrt, bias=eps_t, scale=invd_t)
                rsq128 = small.tile([128, 16], f32, tag="rsq")
                nc.vector.reciprocal_approx_fast(out=rsq128, in_=stdq)
                nc.sync.dma_start(
                    out=rsq_dram.rearrange("(t p) -> p t", p=128), in_=rsq128
                )
                rs_row = singles.tile([1, 2048], f32)
                nc.sync.dma_start(
                    out=rs_row, in_=rsq_dram.rearrange("(a n) -> a n", a=1)
                )
                # q finalize: qT[d, n] *= rstd_q[n] via ones outer-product bcast
                for nb in range(4):
                    bcq = projps.tile([128, 512], f32, tag="proj")
                    nc.tensor.matmul(
                        bcq,
                        ones1x128,
                        rs_row[:, nb * 512 : nb * 512 + 512],
                        start=True,
                        stop=True,
                    )
                    for dt in range(2):
                        nc.vector.tensor_mul(qT[:, dt, nb, :], qT[:, dt, nb, :], bcq)

            # ---- attention: flat stream over g = (pass, mt), pipelined
            # depth-2 ACROSS pass boundaries.  PE order:
            #   ... PV(g-1), QK(g+1), filler, PV(g), QK(g+2), ...
            # ACT order: exp(0), exp(1), ...  The PE never waits on an exp
            # (QK(g+1)'s s2 slot was freed by exp(g-1), one full exp ago) so
            # the HAM clock-gate stays warm.
            # PSUM banks: s2 2x2 + o2 2 + filler/bc/dummy 2x1 = 8.
            with (
                tc.tile_pool(name="sps", bufs=2, space="PSUM") as spool,
                tc.tile_pool(name="ops", bufs=1, space="PSUM") as opool,
                tc.tile_pool(name="fps", bufs=2, space="PSUM") as fpool,
            ):
                passes = [(nbp, h) for nbp in range(2) for h in range(4)]
                G = len(passes) * n_mt
                s2g, p2g, o2cur = {}, {}, {}
                filler = []
                prev = [None]

                def emit_dummy():
                    dum = fpool.tile([128, 512], f32, tag="f")
                    nc.tensor.matmul(
                        dum, kT[:, 0, 0:128], kT[:, 0, 512:1024],
                        start=True, stop=True, skip_group_check=True,
                    )

                def make_outproj(nbp):
                    """one closure per (nb, ot) psum tile: 2 matmuls + copy + dma"""
                    fns = []
                    for nb in (2 * nbp, 2 * nbp + 1):
                        for ot in range(8):
                            def f(nb=nb, ot=ot):
                                ps = fpool.tile([128, 512], f32, tag="f")
                                for dc in range(2):
                                    nc.tensor.matmul(
                                        ps,
                                        wo_sb[:, dc, ot * 128 : ot * 128 + 128],
                                        xT[:, dc, nb, :],
                                        start=(dc == 0),
                                        stop=(dc == 1),
                                    )
                                out_sb = outbuf.tile([128, 512], f32, tag="osb")
                                nc.vector.tensor_copy(out_sb, ps)
                                nc.sync.dma_start(
                                    out=outT_d[
                                        ot * 128 : ot * 128 + 128,
                                        nb * 512 : nb * 512 + 512,
                                    ],
                                    in_=out_sb,
                                )
                            fns.append(f)
                    return fns

                def emit_normalize(state):
                    """bc outer-products + muls for a pass whose DVE recips are
                    done by now (emitted one pass late to keep PE gapless)."""
                    hh, nbp_, oo_sb, rds_ = state
                    ddt, ooff = hh // 2, (hh % 2) * C
                    for i, nb in enumerate((2 * nbp_, 2 * nbp_ + 1)):
                        bc = fpool.tile([128, 512], f32, tag="f")
                        nc.tensor.matmul(
                            bc[0:C, :], ones1x64, rds_[i], start=True, stop=True
                        )
                        nc.vector.tensor_mul(
                            xT[ooff : ooff + C, ddt, nb, :],
                            oo_sb[0:C, i, :],
                            bc[0:C, :],
                        )

                def emit_qk(g):
                    (nbp, h), mt = passes[g // n_mt], g % n_mt
                    dt, off = h // 2, (h % 2) * C
                    s2 = spool.tile([128, 2, 512], f32, tag="s2")
                    kT_lhs = kT[off : off + C, dt, mt * 128 : mt * 128 + 128]
                    for i, nb in enumerate((2 * nbp, 2 * nbp + 1)):
                        nc.tensor.matmul(
                            s2[:, i, :],
                            kT_lhs,
                            qT[off : off + C, dt, nb, :],
                            start=True,
                            stop=True,
                        )
                    s2g[g] = s2

                def emit_exp(g):
                    mt = g % n_mt
                    p2 = ppool.tile([128, 2, 512], f32r, tag="p")
                    nc.scalar.activation(
                        p2, s2g.pop(g), AF.Exp,
                        bias=mb_sb[:, mt : mt + 1],
                        scale=rstdk[:, mt : mt + 1],
                    )
                    p2g[g] = p2

                def emit_pv(g):
                    pi, mt = g // n_mt, g % n_mt
                    nbp, h = passes[pi]
                    if mt == 0:
                        o2_t = opool.tile([C + 1, 2, 512], f32, tag="o2")
                        o2cur[pi] = o2_t
                    o2 = o2cur[pi]
                    p2 = p2g.pop(g)
                    for i in range(2):
                        nc.tensor.matmul(
                            o2[:, i, :],
                            v_sb[:, mt, h, :],
                            p2[:, i, :],
                            start=(mt == 0),
                            stop=(mt == n_mt - 1),
                            skip_group_check=True,
                        )
                    if mt == n_mt - 1:
                        # pass drain: free o2 with one DVE copy; recips; the
                        # PREVIOUS pass's normalize (its recips are done)
                        o_sb = obuf.tile([C + 1, 2, 512], f32, tag="osb")
                        nc.vector.tensor_copy(o_sb, o2cur.pop(pi))
                        rds = []
                        for i in range(2):
                            den = rdp.tile([1, 512], f32, tag="den")
                            nc.vector.tensor_copy(den, o_sb[C : C + 1, i, :])
                            rd = rdp.tile([1, 512], f32, tag="rd")
                            nc.vector.reciprocal_approx_fast(out=rd, in_=den)
                            rds.append(rd)
                        if prev[0] is not None:
                            emit_normalize(prev[0])
                            if prev[0][1] == 0 and prev[0][0] == 3:
                                # nbp=0 fully normalized -> its out-projection
                                # becomes PE filler for the remaining passes
                                filler.extend(make_outproj(0))
                        prev[0] = (h, nbp, o_sb, rds)

                emit_qk(0)
                emit_qk(1)
                rewarm = fpool.tile([128, 512], f32, tag="f")
                for i in range(14):
                    nc.tensor.matmul(
                        rewarm, kT[:, 0, 0:128], kT[:, 0, 512:1024],
                        start=(i == 0), stop=(i == 13), skip_group_check=True,
                    )
                emit_exp(0)
                for _ in range(2):
                    emit_dummy()
                for g in range(1, G):
                    emit_pv(g - 1)
                    if g + 1 < G:
                        emit_qk(g + 1)
                    if filler:
                        filler.pop(0)()
                    else:
                        emit_dummy()
                    emit_exp(g)
                emit_pv(G - 1)

                emit_normalize(prev[0])
                # tail: whatever filler wasn't consumed + second-half out proj
                for f in filler:
                    f()
                for f in make_outproj(1):
                    f()

    nc.finalize()
    return nc


_NC_CACHE = {}


def _get_nc(n_mt: int):
    if n_mt not in _NC_CACHE:
        _NC_CACHE[n_mt] = build(n_mt)
    return _NC_CACHE[n_mt]


def n_mt_for(mask) -> int:
    mask = np.asarray(mask)
    cnt = int(max((mask[b] != 0).sum() for b in range(B)))
    return max(1, (cnt + 127) // 128)


def make_in_maps(querys, key_feats, mask, Wq, Wk, Wv, gq, gk, Wo, bo, n_mt):
    M = n_mt * 128
    querys = np.asarray(querys, dtype=np.float32)
    key_feats = np.asarray(key_feats, dtype=np.float32)
    mask = np.asarray(mask)
    gq = np.asarray(gq, dtype=np.float32)
    gk = np.asarray(gk, dtype=np.float32)

    gsq_full = gq * np.float32(SCALE)  # folded into Wq rows
    gsk_full = gk.astype(np.float32)  # folded into Wk rows
    Wq_f = np.asarray(Wq, dtype=np.float32) * gsq_full[:, None]
    Wk_f = np.asarray(Wk, dtype=np.float32) * gsk_full[:, None]

    qT = [round_f32r(querys[b].T) for b in range(B)]
    kfT, mb = [], []
    for b in range(B):
        idx = np.flatnonzero(mask[b] != 0)
        cnt = len(idx)
        kc = np.zeros((M, E), np.float32)
        kc[:cnt] = key_feats[b][idx]
        kfT.append(round_f32r(kc.T))
        mbv = np.full((M,), NEG, dtype=np.float32)
        mbv[:cnt] = 0.0
        mb.append(mbv.reshape(n_mt, 128))

    wqT, wkT, wvT, woT, ig2q, ig2k = [], [], [], [], [], []
    for j in range(4):
        dsl = slice(j * DS, (j + 1) * DS)
        wqT.append(round_f32r(Wq_f[dsl].T))
        wkT.append(round_f32r(Wk_f[dsl].T))
        wvT.append(round_f32r(np.asarray(Wv)[dsl].T))
        woT.append(round_f32r(np.asarray(Wo)[:, dsl].T))
        # sumsq compensation: raw sumsq = sum_d (q'_d)^2 / gs_d^2
        ig2q.append(round_f32r((1.0 / gsq_full[dsl] ** 2).reshape(2, 128)))
        ig2k.append(round_f32r((1.0 / gsk_full[dsl] ** 2).reshape(2, 128)))

    in_maps = []
    for cid in range(NCORES):
        b, j = cid // 4, cid % 4
        in_maps.append(
            {
                "qT": qT[b],
                "kfT": kfT[b],
                "wqT": wqT[j],
                "wkT": wkT[j],
                "wvT": wvT[j],
                "woT": woT[j],
                "ig2q": ig2q[j],
                "ig2k": ig2k[j],
                "mbias": mb[b],
            }
        )
    return in_maps


def assemble(results, bo):
    bo = np.asarray(bo, dtype=np.float32)
    out = np.zeros((B, N, D), dtype=np.float32)
    for cid in range(NCORES):
        b = cid // 4
        out[b] += results[cid]["outT"].T
    out += bo
    return out


def kernel(querys, key_feats, mask, Wq, Wk, Wv, gq, gk, Wo, bo):
    n_mt = n_mt_for(mask)
    nc = _get_nc(n_mt)
    in_maps = make_in_maps(querys, key_feats, mask, Wq, Wk, Wv, gq, gk, Wo, bo, n_mt)
    res = run_bass_kernel_spmd(nc, in_maps, list(range(NCORES)))
    return assemble(res.results, bo)


# revision 16
# speedup vs baseline: 1.0220x; 1.0142x over previous
"""CrossAttention Trainium2 kernel (8 NeuronCores).

Reference computation (B=2, N=M=2048, D=1024, H=16, C=64):
    q = rmsnorm(querys @ Wq.T, gq) * C**-0.5       [B,N,D]
    k = rmsnorm(key_feats @ Wk.T, gk)              [B,M,D]
    v = key_feats @ Wv.T                           [B,M,D]
    attn = softmax(mask(q @ k.T per head))         [B,H,N,M]
    out = (attn @ v per head, concat) @ Wo.T + bo  [B,N,D]

Sharding: core = b*4 + j (b in {0,1}; j in {0..3} owns heads 4j..4j+3 = a
256-wide slice of D). Host pre-transposes inputs/weights, folds gq*scale /
gk into Wq / Wk rows, and pre-rounds everything to f32r (fp32 with 11-bit
mantissa -> full PE rate).

v2 structural changes vs v1:
  - Mask compaction: rows with mask==0 contribute exp(-inf)=0 to both the
    softmax denominator and PV, so the host gathers only the valid kf
    columns (per batch), pads to a multiple of 128, and the kernel runs
    with M_pad ~= 1152 instead of 2048.  All M-side work (k/v projection,
    QK, exp, PV, kf DMA) shrinks ~2x.  Padding columns carry bias -1e30
    into the exp -> contribute exactly 0.
  - k and v projections fused over a single kfT stream (halves kf DMA).
  - One fused AllReduce carries both q and k partial sum-of-squares
    (2048 + M_pad floats); its ~27us mesh latency is bridged by a long
    dependency-free dummy-matmul burst that keeps the PE HAM clock-gate
    warm (K=8/8) into attention.
  - Attention is software-pipelined depth-2 per (nbp, h) pass: PE order is
    ... PV(mt-1), QK(mt+1), filler, PV(mt) ... so the PE never stalls on
    the ACT exp (v1 stalled ~0.4us every mt, which kept HAM at K=4/8 =
    1.2 GHz for the whole 314us attention phase).  ACT exp (~1.2us/mt) is
    the pace-setter; PE real work is ~1.0us/mt, padded by a dummy matmul
    (first n-half) or an out-projection matmul pair (second n-half).
  - The out projection (partial over this core's d-slice) is interleaved
    into attention as filler work; the host sums 4 partials per b and
    adds bo.
"""

import numpy as np

import concourse.tile as tile
from concourse import bacc, mybir
from concourse.bass_utils import run_bass_kernel_spmd

B, N, M_FULL, D, H = 2, 2048, 2048, 1024, 16
C = D // H  # 64, head dim
E = D  # input feature dim
EPS = 1e-6
SCALE = C ** (-0.5)
DS = D // 4  # 256, per-core d-slice
NCORES = 8

f32 = mybir.dt.float32
f32r = mybir.dt.float32r
bf16 = mybir.dt.bfloat16
AF = mybir.ActivationFunctionType

NEG = -1e30
WARM_MM = 60  # dummy matmuls bridging the AllReduce latency into attention


def round_f32r(x: np.ndarray) -> np.ndarray:
    b = np.ascontiguousarray(x, dtype=np.float32).view(np.uint32)
    b = (b + 0x800) & np.uint32(0xFFFFF000)
    return b.view(np.float32)


def build(n_mt: int):
    M = n_mt * 128
    mblocks = []
    off = 0
    while off < M:
        w = min(512, M - off)
        mblocks.append((off, w))
        off += w

    nc = bacc.Bacc(None, target_bir_lowering=False)

    qT_d = nc.declare_dram_parameter("qT", [E, N], f32r, isOutput=False)
    kfT_d = nc.declare_dram_parameter("kfT", [E, M], f32r, isOutput=False)
    wqT_d = nc.declare_dram_parameter("wqT", [E, DS], f32r, isOutput=False)
    wkT_d = nc.declare_dram_parameter("wkT", [E, DS], f32r, isOutput=False)
    wvT_d = nc.declare_dram_parameter("wvT", [E, DS], f32r, isOutput=False)
    woT_d = nc.declare_dram_parameter("woT", [DS, D], f32r, isOutput=False)
    ig2q_d = nc.declare_dram_parameter("ig2q", [2, 128], f32r, isOutput=False)
    ig2k_d = nc.declare_dram_parameter("ig2k", [2, 128], f32r, isOutput=False)
    mb_d = nc.declare_dram_parameter("mbias", [n_mt, 128], f32, isOutput=False)
    outT_d = nc.declare_dram_parameter("outT", [D, N], f32, isOutput=True)

    with (
        nc.allow_low_precision(reason="f32r matmul operands by design; fp32 PSUM"),
        tile.TileContext(nc) as tc,
    ):
        with (
            tc.tile_pool(name="singles", bufs=1) as singles,
            tc.tile_pool(name="wts", bufs=3) as wts,
            tc.tile_pool(name="blk", bufs=2) as blkpool,
            tc.tile_pool(name="sq", bufs=2) as sqpool,
            tc.tile_pool(name="psb", bufs=3) as ppool,
            tc.tile_pool(name="obuf", bufs=2) as obuf,
            tc.tile_pool(name="osb2", bufs=10) as outbuf,
            tc.tile_pool(name="rdp", bufs=4) as rdp,
            tc.tile_pool(name="small", bufs=2) as small,
            tc.tile_pool(name="dram", bufs=1, space="DRAM") as dram,
        ):
            # ---- constants / small inputs ----
            ones1x64 = singles.tile([1, 64], f32)
            nc.vector.memset(ones1x64, 1.0)
            ones1x128 = singles.tile([1, 128], f32)
            nc.vector.memset(ones1x128, 1.0)
            onesv = singles.tile([128, n_mt * 4], f32)
            nc.vector.memset(onesv, 1.0)
            eps_t = singles.tile([128, 1], f32)
            nc.vector.memset(eps_t, EPS)
            invd_t = singles.tile([128, 1], f32)
            nc.vector.memset(invd_t, 1.0 / D)
            ig2q_sb = singles.tile([128, 2], f32r)
            nc.sync.dma_start(out=ig2q_sb, in_=ig2q_d.rearrange("t p -> p t"))
            ig2k_sb = singles.tile([128, 2], f32r)
            nc.sync.dma_start(out=ig2k_sb, in_=ig2k_d.rearrange("t p -> p t"))
            mb_sb = singles.tile([128, n_mt], f32)
            nc.sync.dma_start(out=mb_sb, in_=mb_d.rearrange("t p -> p t"))

            # weights: wq, wk, wv upfront; wo reuses wq's slot after q proj
            wq_sb = wts.tile([128, 8, DS], f32r, tag="w")
            wk_sb = wts.tile([128, 8, DS], f32r, tag="w")
            wv_sb = wts.tile([128, 8, DS], f32r, tag="w")
            for et in range(8):
                nc.sync.dma_start(out=wq_sb[:, et, :], in_=wqT_d[et * 128 : et * 128 + 128, :])
                nc.scalar.dma_start(out=wk_sb[:, et, :], in_=wkT_d[et * 128 : et * 128 + 128, :])
                nc.scalar.dma_start(out=wv_sb[:, et, :], in_=wvT_d[et * 128 : et * 128 + 128, :])

            # ---- persistent activations ----
            qT = singles.tile([128, 2, 4, 512], f32r)  # [p, dt, nb, n]
            kT = singles.tile([128, 2, M], f32r)  # [p, dt, m]
            v_sb = singles.tile([128, n_mt, 4, C + 1], bf16)  # [m_p, mt, h, c|ones]
            xT = singles.tile([128, 2, 4, 512], f32r)  # [p, dt, nb, n]
            nc.vector.tensor_copy(
                v_sb[:, :, :, C], onesv.rearrange("p (a b) -> p a b", a=n_mt)
            )

            ccq_in = dram.tile([2048], f32)
            ccq_out = dram.tile([2048], f32)
            cck_in = dram.tile([M], f32)
            cck_out = dram.tile([M], f32)
            rsq_dram = dram.tile([2048], f32)

            with (
                tc.tile_pool(name="projps", bufs=2, space="PSUM") as projps,
                tc.tile_pool(name="vps", bufs=2, space="PSUM") as vps,
                tc.tile_pool(name="ssps", bufs=2, space="PSUM") as ssps,
            ):
                # ---- fused k+v projection over one kfT stream ----
                for moff, w in mblocks:
                    blk = blkpool.tile([128, 8, 512], f32r, tag="blk")
                    for et in range(8):
                        nc.scalar.dma_start(
                            out=blk[:, et, 0:w],
                            in_=kfT_d[et * 128 : et * 128 + 128, moff : moff + w],
                        )
                    ss_ps = ssps.tile([1, 512], f32, tag="ss")
                    for dt in range(2):
                        ps = projps.tile([128, 512], f32, tag="proj")
                        for et in range(8):
                            nc.tensor.matmul(
                                ps[:, 0:w],
                                wk_sb[:, et, dt * 128 : dt * 128 + 128],
                                blk[:, et, 0:w],
                                start=(et == 0),
                                stop=(et == 7),
                            )
                        nc.vector.tensor_copy(kT[:, dt, moff : moff + w], ps[:, 0:w])
                        sq = sqpool.tile([128, 512], f32r, tag="sq")
                        nc.vector.tensor_mul(
                            sq[:, 0:w],
                            kT[:, dt, moff : moff + w],
                            kT[:, dt, moff : moff + w],
                        )
                        nc.tensor.matmul(
                            ss_ps[:, 0:w],
                            ig2k_sb[:, dt : dt + 1],
                            sq[:, 0:w],
                            start=(dt == 0),
                            stop=(dt == 1),
                            skip_group_check=True,
                        )
                    ss_sb = small.tile([1, 512], f32, tag="ss_sb")
                    nc.scalar.copy(ss_sb[:, 0:w], ss_ps[:, 0:w])
                    nc.sync.dma_start(
                        out=cck_in[moff : moff + w].rearrange(
                            "(a n) -> a n", a=1
                        ),
                        in_=ss_sb[:, 0:w],
                    )
                    # v from the same resident block
                    for ct in range(w // 128):
                        mtg = moff // 128 + ct
                        psv = vps.tile([128, 256], f32, tag="v")
                        for et in range(8):
                            nc.tensor.matmul(
                                psv,
                                blk[:, et, ct * 128 : ct * 128 + 128],
                                wv_sb[:, et, :],
                                start=(et == 0),
                                stop=(et == 7),
                            )
                        nc.vector.tensor_copy(
                            v_sb[:, mtg, :, 0:C],
                            psv.rearrange("p (h c) -> p h c", c=C),
                        )

                nc.gpsimd.collective_compute(
                    "AllReduce",
                    mybir.AluOpType.add,
                    replica_groups=[[0, 1, 2, 3], [4, 5, 6, 7]],
                    ins=[cck_in.opt()],
                    outs=[cck_out.opt()],
                )

                # ---- q projection: qT[dt, nb] = Wq'^T-slice @ q-block ----
                for nb in range(4):
                    blk = blkpool.tile([128, 8, 512], f32r, tag="blk")
                    for et in range(8):
                        nc.sync.dma_start(
                            out=blk[:, et, :],
                            in_=qT_d[et * 128 : et * 128 + 128, nb * 512 : nb * 512 + 512],
                        )
                    ss_ps = ssps.tile([1, 512], f32, tag="ss")
                    for dt in range(2):
                        ps = projps.tile([128, 512], f32, tag="proj")
                        for et in range(8):
                            nc.tensor.matmul(
                                ps,
                                wq_sb[:, et, dt * 128 : dt * 128 + 128],
                                blk[:, et, :],
                                start=(et == 0),
                                stop=(et == 7),
                            )
                        nc.vector.tensor_copy(qT[:, dt, nb, :], ps)
                        sq = sqpool.tile([128, 512], f32r, tag="sq")
                        nc.vector.tensor_mul(sq, qT[:, dt, nb, :], qT[:, dt, nb, :])
                        nc.tensor.matmul(
                            ss_ps,
                            ig2q_sb[:, dt : dt + 1],
                            sq,
                            start=(dt == 0),
                            stop=(dt == 1),
                            skip_group_check=True,
                        )
                    ss_sb = small.tile([1, 512], f32, tag="ss_sb")
                    nc.scalar.copy(ss_sb, ss_ps)
                    nc.sync.dma_start(
                        out=ccq_in[nb * 512 : nb * 512 + 512].rearrange(
                            "(a n) -> a n", a=1
                        ),
                        in_=ss_sb,
                    )

                # AR_q fires while the kv projection runs; AR_k pipelines
                # behind it on the CC queue.
                nc.gpsimd.collective_compute(
                    "AllReduce",
                    mybir.AluOpType.add,
                    replica_groups=[[0, 1, 2, 3], [4, 5, 6, 7]],
                    ins=[ccq_in.opt()],
                    outs=[ccq_out.opt()],
                )

                # wo weight load (DMA only; hides under the AllReduce)
                wo_sb = wts.tile([128, 2, D], f32r, tag="w")
                for dc in range(2):
                    nc.sync.dma_start(
                        out=wo_sb[:, dc, :], in_=woT_d[dc * 128 : dc * 128 + 128, :]
                    )

                # ---- warm burst: dependency-free matmuls spanning the
                # AllReduce latency so the PE HAM clock-gate stays at 8/8
                # when attention starts (a PE-idle window here would drop it
                # to 4/8 and the attention stream would start cold) ----
                warm = projps.tile([128, 512], f32, tag="proj")
                for i in range(WARM_MM):
                    nc.tensor.matmul(
                        warm,
                        kT[:, 0, 0:128],
                        kT[:, 0, 512:1024],
                        start=(i == 0),
                        stop=(i == WARM_MM - 1),
                        skip_group_check=True,
                    )
                warm_sink = small.tile([1, 512], f32, tag="rd")
                nc.vector.tensor_copy(warm_sink, warm[0:1, :])

                # ---- rstd_k: [128, n_mt] lane-parallel; feeds exp scale ----
                ss128 = small.tile([128, n_mt], f32, tag="ssk")
                nc.sync.dma_start(
                    out=ss128, in_=cck_out.rearrange("(t p) -> p t", p=128)
                )
                stdk = small.tile([128, n_mt], f32, tag="stdk")
                nc.scalar.activation(stdk, ss128, AF.Sqrt, bias=eps_t, scale=invd_t)
                rstdk = singles.tile([128, n_mt], f32)
                nc.vector.reciprocal_approx_fast(out=rstdk, in_=stdk)

                # ---- rstd_q: lane-parallel [128, 16], then a DRAM bounce to
                # the [1, 2048] row layout the bcast outer-product wants
                # (single-lane sqrt/recip on [1, 2048] costs ~5us; this ~1us)
                ssq128 = small.tile([128, 16], f32, tag="ssq")
                nc.sync.dma_start(
                    out=ssq128, in_=ccq_out.rearrange("(t p) -> p t", p=128)
                )
                stdq = small.tile([128, 16], f32, tag="stdq")
                nc.scalar.activation(stdq, ssq128, AF.Sqrt, bias=eps_t, scale=invd_t)
                rsq128 = small.tile([128, 16], f32, tag="rsq")
                nc.vector.reciprocal_approx_fast(out=rsq128, in_=stdq)
                nc.sync.dma_start(
                    out=rsq_dram.rearrange("(t p) -> p t", p=128), in_=rsq128
                )
                rs_row = singles.tile([1, 2048], f32)
                nc.sync.dma_start(
                    out=rs_row, in_=rsq_dram.rearrange("(a n) -> a n", a=1)
                )
                # q finalize: qT[d, n] *= rstd_q[n] via ones outer-product bcast
                for nb in range(4):
                    bcq = projps.tile([128, 512], f32, tag="proj")
                    nc.tensor.matmul(
                        bcq,
                        ones1x128,
                        rs_row[:, nb * 512 : nb * 512 + 512],
                        start=True,
                        stop=True,
                    )
                    for dt in range(2):
                        nc.vector.tensor_mul(qT[:, dt, nb, :], qT[:, dt, nb, :], bcq)

            # ---- attention: flat stream over g = (pass, mt), pipelined
            # depth-2 ACROSS pass boundaries.  PE order:
            #   ... PV(g-1), QK(g+1), filler, PV(g), QK(g+2), ...
            # ACT order: exp(0), exp(1), ...  The PE never waits on an exp
            # (QK(g+1)'s s2 slot was freed by exp(g-1), one full exp ago) so
            # the HAM clock-gate stays warm.
            # PSUM banks: s2 2x2 + o2 2 + filler/bc/dummy 2x1 = 8.
            with (
                tc.tile_pool(name="sps", bufs=2, space="PSUM") as spool,
                tc.tile_pool(name="ops", bufs=1, space="PSUM") as opool,
                tc.tile_pool(name="fps", bufs=2, space="PSUM") as fpool,
            ):
                passes = [(nbp, h) for nbp in range(2) for h in range(4)]
                G = len(passes) * n_mt
                s2g, p2g, o2cur = {}, {}, {}
                filler = []
                prev = [None]

                def emit_dummy():
                    dum = fpool.tile([128, 512], f32, tag="f")
                    nc.tensor.matmul(
                        dum, kT[:, 0, 0:128], kT[:, 0, 512:1024],
                        start=True, stop=True, skip_group_check=True,
                    )

                def make_outproj(nbp):
                    """one closure per (nb, ot) psum tile: 2 matmuls + copy + dma"""
                    fns = []
                    for nb in (2 * nbp, 2 * nbp + 1):
                        for ot in range(8):
                            def f(nb=nb, ot=ot):
                                ps = fpool.tile([128, 512], f32, tag="f")
                                for dc in range(2):
                                    nc.tensor.matmul(
                                        ps,
                                        wo_sb[:, dc, ot * 128 : ot * 128 + 128],
                                        xT[:, dc, nb, :],
                                        start=(dc == 0),
                                        stop=(dc == 1),
                                    )
                                out_sb = outbuf.tile([128, 512], f32, tag="osb")
                                nc.vector.tensor_copy(out_sb, ps)
                                nc.sync.dma_start(
                                    out=outT_d[
                                        ot * 128 : ot * 128 + 128,
                                        nb * 512 : nb * 512 + 512,
                                    ],
                                    in_=out_sb,
                                )
                            fns.append(f)
                    return fns

                def emit_normalize(state):
                    """bc outer-products + muls for a pass whose DVE recips are
                    done by now (emitted one pass late to keep PE gapless)."""
                    hh, nbp_, oo_sb, rds_ = state
                    ddt, ooff = hh // 2, (hh % 2) * C
                    for i, nb in enumerate((2 * nbp_, 2 * nbp_ + 1)):
                        bc = fpool.tile([128, 512], f32, tag="f")
                        nc.tensor.matmul(
                            bc[0:C, :], ones1x64, rds_[i], start=True, stop=True
                        )
                        nc.vector.tensor_mul(
                            xT[ooff : ooff + C, ddt, nb, :],
                            oo_sb[0:C, i, :],
                            bc[0:C, :],
                        )

                def emit_qk(g):
                    (nbp, h), mt = passes[g // n_mt], g % n_mt
                    dt, off = h // 2, (h % 2) * C
                    s2 = spool.tile([128, 2, 512], f32, tag="s2")
                    kT_lhs = kT[off : off + C, dt, mt * 128 : mt * 128 + 128]
                    for i, nb in enumerate((2 * nbp, 2 * nbp + 1)):
                        nc.tensor.matmul(
                            s2[:, i, :],
                            kT_lhs,
                            qT[off : off + C, dt, nb, :],
                            start=True,
                            stop=True,
                        )
                    s2g[g] = s2

                def emit_exp(g):
                    mt = g % n_mt
                    p2 = ppool.tile([128, 2, 512], bf16, tag="p")
                    nc.scalar.activation(
                        p2, s2g.pop(g), AF.Exp,
                        bias=mb_sb[:, mt : mt + 1],
                        scale=rstdk[:, mt : mt + 1],
                    )
                    p2g[g] = p2

                def emit_pv(g):
                    pi, mt = g // n_mt, g % n_mt
                    nbp, h = passes[pi]
                    if mt == 0:
                        o2_t = opool.tile([C + 1, 2, 512], f32, tag="o2")
                        o2cur[pi] = o2_t
                    o2 = o2cur[pi]
                    p2 = p2g.pop(g)
                    for i in range(2):
                        nc.tensor.matmul(
                            o2[:, i, :],
                            v_sb[:, mt, h, :],
                            p2[:, i, :],
                            start=(mt == 0),
                            stop=(mt == n_mt - 1),
                            skip_group_check=True,
                        )
                    if mt == n_mt - 1:
                        # pass drain: free o2 with one DVE copy; recips; the
                        # PREVIOUS pass's normalize (its recips are done)
                        o_sb = obuf.tile([C + 1, 2, 512], f32, tag="osb")
                        nc.vector.tensor_copy(o_sb, o2cur.pop(pi))
                        rds = []
                        for i in range(2):
                            den = rdp.tile([1, 512], f32, tag="den")
                            nc.vector.tensor_copy(den, o_sb[C : C + 1, i, :])
                            rd = rdp.tile([1, 512], f32, tag="rd")
                            nc.vector.reciprocal_approx_fast(out=rd, in_=den)
                            rds.append(rd)
                        if prev[0] is not None:
                            emit_normalize(prev[0])
                            if prev[0][1] == 0 and prev[0][0] == 3:
                                # nbp=0 fully normalized -> its out-projection
                                # becomes PE filler for the remaining passes
                                filler.extend(make_outproj(0))
                        prev[0] = (h, nbp, o_sb, rds)

                emit_qk(0)
                emit_qk(1)
                rewarm = fpool.tile([128, 512], f32, tag="f")
                for i in range(14):
                    nc.tensor.matmul(
                        rewarm, kT[:, 0, 0:128], kT[:, 0, 512:1024],
                        start=(i == 0), stop=(i == 13), skip_group_check=True,
                    )
                emit_exp(0)
                for _ in range(2):
                    emit_dummy()
                for g in range(1, G):
                    emit_pv(g - 1)
                    if g + 1 < G:
                        emit_qk(g + 1)
                    if filler:
                        filler.pop(0)()
                    else:
                        emit_dummy()
                    emit_exp(g)
                emit_pv(G - 1)

                emit_normalize(prev[0])
                # tail: whatever filler wasn't consumed + second-half out proj
                for f in filler:
                    f()
                for f in make_outproj(1):
                    f()

    nc.finalize()
    return nc


_NC_CACHE = {}


def _get_nc(n_mt: int):
    if n_mt not in _NC_CACHE:
        _NC_CACHE[n_mt] = build(n_mt)
    return _NC_CACHE[n_mt]


def n_mt_for(mask) -> int:
    mask = np.asarray(mask)
    cnt = int(max((mask[b] != 0).sum() for b in range(B)))
    return max(1, (cnt + 127) // 128)


def make_in_maps(querys, key_feats, mask, Wq, Wk, Wv, gq, gk, Wo, bo, n_mt):
    M = n_mt * 128
    querys = np.asarray(querys, dtype=np.float32)
    key_feats = np.asarray(key_feats, dtype=np.float32)
    mask = np.asarray(mask)
    gq = np.asarray(gq, dtype=np.float32)
    gk = np.asarray(gk, dtype=np.float32)

    gsq_full = gq * np.float32(SCALE)  # folded into Wq rows
    gsk_full = gk.astype(np.float32)  # folded into Wk rows
    Wq_f = np.asarray(Wq, dtype=np.float32) * gsq_full[:, None]
    Wk_f = np.asarray(Wk, dtype=np.float32) * gsk_full[:, None]

    qT = [round_f32r(querys[b].T) for b in range(B)]
    kfT, mb = [], []
    for b in range(B):
        idx = np.flatnonzero(mask[b] != 0)
        cnt = len(idx)
        kc = np.zeros((M, E), np.float32)
        kc[:cnt] = key_feats[b][idx]
        kfT.append(round_f32r(kc.T))
        mbv = np.full((M,), NEG, dtype=np.float32)
        mbv[:cnt] = 0.0
        mb.append(mbv.reshape(n_mt, 128))

    wqT, wkT, wvT, woT, ig2q, ig2k = [], [], [], [], [], []
    for j in range(4):
        dsl = slice(j * DS, (j + 1) * DS)
        wqT.append(round_f32r(Wq_f[dsl].T))
        wkT.append(round_f32r(Wk_f[dsl].T))
        wvT.append(round_f32r(np.asarray(Wv)[dsl].T))
        woT.append(round_f32r(np.asarray(Wo)[:, dsl].T))
        # sumsq compensation: raw sumsq = sum_d (q'_d)^2 / gs_d^2
        ig2q.append(round_f32r((1.0 / gsq_full[dsl] ** 2).reshape(2, 128)))
        ig2k.append(round_f32r((1.0 / gsk_full[dsl] ** 2).reshape(2, 128)))

    in_maps = []
    for cid in range(NCORES):
        b, j = cid // 4, cid % 4
        in_maps.append(
            {
                "qT": qT[b],
                "kfT": kfT[b],
                "wqT": wqT[j],
                "wkT": wkT[j],
                "wvT": wvT[j],
                "woT": woT[j],
                "ig2q": ig2q[j],
                "ig2k": ig2k[j],
                "mbias": mb[b],
            }
        )
    return in_maps


def assemble(results, bo):
    bo = np.asarray(bo, dtype=np.float32)
    out = np.zeros((B, N, D), dtype=np.float32)
    for cid in range(NCORES):
        b = cid // 4
        out[b] += results[cid]["outT"].T
    out += bo
    return out


def kernel(querys, key_feats, mask, Wq, Wk, Wv, gq, gk, Wo, bo):
    n_mt = n_mt_for(mask)
    nc = _get_nc(n_mt)
    in_maps = make_in_maps(querys, key_feats, mask, Wq, Wk, Wv, gq, gk, Wo, bo, n_mt)
    res = run_bass_kernel_spmd(nc, in_maps, list(range(NCORES)))
    return assemble(res.results, bo)


# revision 17
# speedup vs baseline: 1.0831x; 1.0598x over previous
"""CrossAttention Trainium2 kernel (8 NeuronCores).

Reference computation (B=2, N=M=2048, D=1024, H=16, C=64):
    q = rmsnorm(querys @ Wq.T, gq) * C**-0.5       [B,N,D]
    k = rmsnorm(key_feats @ Wk.T, gk)              [B,M,D]
    v = key_feats @ Wv.T                           [B,M,D]
    attn = softmax(mask(q @ k.T per head))         [B,H,N,M]
    out = (attn @ v per head, concat) @ Wo.T + bo  [B,N,D]

Sharding: core = b*4 + j (b in {0,1}; j in {0..3} owns heads 4j..4j+3 = a
256-wide slice of D). Host pre-transposes inputs/weights, folds gq*scale /
gk into Wq / Wk rows, and pre-rounds everything to f32r (fp32 with 11-bit
mantissa -> full PE rate).

v2 structural changes vs v1:
  - Mask compaction: rows with mask==0 contribute exp(-inf)=0 to both the
    softmax denominator and PV, so the host gathers only the valid kf
    columns (per batch), pads to a multiple of 128, and the kernel runs
    with M_pad ~= 1152 instead of 2048.  All M-side work (k/v projection,
    QK, exp, PV, kf DMA) shrinks ~2x.  Padding columns carry bias -1e30
    into the exp -> contribute exactly 0.
  - k and v projections fused over a single kfT stream (halves kf DMA).
  - One fused AllReduce carries both q and k partial sum-of-squares
    (2048 + M_pad floats); its ~27us mesh latency is bridged by a long
    dependency-free dummy-matmul burst that keeps the PE HAM clock-gate
    warm (K=8/8) into attention.
  - Attention is software-pipelined depth-2 per (nbp, h) pass: PE order is
    ... PV(mt-1), QK(mt+1), filler, PV(mt) ... so the PE never stalls on
    the ACT exp (v1 stalled ~0.4us every mt, which kept HAM at K=4/8 =
    1.2 GHz for the whole 314us attention phase).  ACT exp (~1.2us/mt) is
    the pace-setter; PE real work is ~1.0us/mt, padded by a dummy matmul
    (first n-half) or an out-projection matmul pair (second n-half).
  - The out projection (partial over this core's d-slice) is interleaved
    into attention as filler work; the host sums 4 partials per b and
    adds bo.
"""

import numpy as np

import concourse.tile as tile
from concourse import bacc, mybir
from concourse.bass_utils import run_bass_kernel_spmd

B, N, M_FULL, D, H = 2, 2048, 2048, 1024, 16
C = D // H  # 64, head dim
E = D  # input feature dim
EPS = 1e-6
SCALE = C ** (-0.5)
DS = D // 4  # 256, per-core d-slice
NCORES = 8

f32 = mybir.dt.float32
f32r = mybir.dt.float32r
bf16 = mybir.dt.bfloat16
AF = mybir.ActivationFunctionType

NEG = -1e30
WARM_MM = 60  # dummy matmuls bridging the AllReduce latency into attention


def round_f32r(x: np.ndarray) -> np.ndarray:
    b = np.ascontiguousarray(x, dtype=np.float32).view(np.uint32)
    b = (b + 0x800) & np.uint32(0xFFFFF000)
    return b.view(np.float32)


def build(n_mt: int):
    M = n_mt * 128
    mblocks = []
    off = 0
    while off < M:
        w = min(512, M - off)
        mblocks.append((off, w))
        off += w

    nc = bacc.Bacc(None, target_bir_lowering=False)

    qT_d = nc.declare_dram_parameter("qT", [E, N], f32r, isOutput=False)
    kfT_d = nc.declare_dram_parameter("kfT", [E, M], f32r, isOutput=False)
    wqT_d = nc.declare_dram_parameter("wqT", [E, DS], f32r, isOutput=False)
    wkT_d = nc.declare_dram_parameter("wkT", [E, DS], f32r, isOutput=False)
    wvT_d = nc.declare_dram_parameter("wvT", [E, DS], f32r, isOutput=False)
    woT_d = nc.declare_dram_parameter("woT", [DS, D], f32r, isOutput=False)
    ig2q_d = nc.declare_dram_parameter("ig2q", [2, 128], f32r, isOutput=False)
    ig2k_d = nc.declare_dram_parameter("ig2k", [2, 128], f32r, isOutput=False)
    mb_d = nc.declare_dram_parameter("mbias", [n_mt, 128], f32, isOutput=False)
    outT_d = nc.declare_dram_parameter("outT", [D, N], f32, isOutput=True)

    with (
        nc.allow_low_precision(reason="f32r matmul operands by design; fp32 PSUM"),
        tile.TileContext(nc) as tc,
    ):
        with (
            tc.tile_pool(name="singles", bufs=1) as singles,
            tc.tile_pool(name="wts", bufs=3) as wts,
            tc.tile_pool(name="blk", bufs=2) as blkpool,
            tc.tile_pool(name="sq", bufs=2) as sqpool,
            tc.tile_pool(name="psb", bufs=3) as ppool,
            tc.tile_pool(name="obuf", bufs=2) as obuf,
            tc.tile_pool(name="osb2", bufs=10) as outbuf,
            tc.tile_pool(name="rdp", bufs=4) as rdp,
            tc.tile_pool(name="small", bufs=2) as small,
            tc.tile_pool(name="dram", bufs=1, space="DRAM") as dram,
        ):
            # ---- constants / small inputs ----
            ones1x64 = singles.tile([1, 64], f32)
            nc.vector.memset(ones1x64, 1.0)
            ones1x128 = singles.tile([1, 128], f32)
            nc.vector.memset(ones1x128, 1.0)
            onesv = singles.tile([128, n_mt * 4], f32)
            nc.vector.memset(onesv, 1.0)
            eps_t = singles.tile([128, 1], f32)
            nc.vector.memset(eps_t, EPS)
            invd_t = singles.tile([128, 1], f32)
            nc.vector.memset(invd_t, 1.0 / D)
            ig2q_sb = singles.tile([128, 2], f32r)
            nc.sync.dma_start(out=ig2q_sb, in_=ig2q_d.rearrange("t p -> p t"))
            ig2k_sb = singles.tile([128, 2], f32r)
            nc.sync.dma_start(out=ig2k_sb, in_=ig2k_d.rearrange("t p -> p t"))
            mb_sb = singles.tile([128, n_mt], f32)
            nc.sync.dma_start(out=mb_sb, in_=mb_d.rearrange("t p -> p t"))

            # weights: wq, wk, wv upfront; wo reuses wq's slot after q proj
            wq_sb = wts.tile([128, 8, DS], f32r, tag="w")
            wk_sb = wts.tile([128, 8, DS], f32r, tag="w")
            wv_sb = wts.tile([128, 8, DS], f32r, tag="w")
            for et in range(8):
                nc.sync.dma_start(out=wq_sb[:, et, :], in_=wqT_d[et * 128 : et * 128 + 128, :])
                nc.scalar.dma_start(out=wk_sb[:, et, :], in_=wkT_d[et * 128 : et * 128 + 128, :])
                nc.scalar.dma_start(out=wv_sb[:, et, :], in_=wvT_d[et * 128 : et * 128 + 128, :])

            # ---- persistent activations ----
            qT = singles.tile([128, 2, 4, 512], f32r)  # [p, dt, nb, n]
            kT = singles.tile([128, 2, M], f32r)  # [p, dt, m]
            v_sb = singles.tile([128, n_mt, 4, C + 1], bf16)  # [m_p, mt, h, c|ones]
            xT = singles.tile([128, 2, 4, 512], f32r)  # [p, dt, nb, n]
            nc.vector.tensor_copy(
                v_sb[:, :, :, C], onesv.rearrange("p (a b) -> p a b", a=n_mt)
            )

            ccq_in = dram.tile([2048], f32)
            ccq_out = dram.tile([2048], f32)
            cck_in = dram.tile([M], f32)
            cck_out = dram.tile([M], f32)
            rsq_dram = dram.tile([2048], f32)

            with (
                tc.tile_pool(name="projps", bufs=2, space="PSUM") as projps,
                tc.tile_pool(name="vps", bufs=2, space="PSUM") as vps,
                tc.tile_pool(name="ssps", bufs=2, space="PSUM") as ssps,
            ):
                # ---- fused k+v projection over one kfT stream ----
                for moff, w in mblocks:
                    blk = blkpool.tile([128, 8, 512], f32r, tag="blk")
                    for et in range(8):
                        nc.scalar.dma_start(
                            out=blk[:, et, 0:w],
                            in_=kfT_d[et * 128 : et * 128 + 128, moff : moff + w],
                        )
                    ss_ps = ssps.tile([1, 512], f32, tag="ss")
                    for dt in range(2):
                        ps = projps.tile([128, 512], f32, tag="proj")
                        for et in range(8):
                            nc.tensor.matmul(
                                ps[:, 0:w],
                                wk_sb[:, et, dt * 128 : dt * 128 + 128],
                                blk[:, et, 0:w],
                                start=(et == 0),
                                stop=(et == 7),
                            )
                        nc.vector.tensor_copy(kT[:, dt, moff : moff + w], ps[:, 0:w])
                        sq = sqpool.tile([128, 512], f32r, tag="sq")
                        nc.vector.tensor_mul(
                            sq[:, 0:w],
                            kT[:, dt, moff : moff + w],
                            kT[:, dt, moff : moff + w],
                        )
                        nc.tensor.matmul(
                            ss_ps[:, 0:w],
                            ig2k_sb[:, dt : dt + 1],
                            sq[:, 0:w],
                            start=(dt == 0),
                            stop=(dt == 1),
                            skip_group_check=True,
                        )
                    ss_sb = small.tile([1, 512], f32, tag="ss_sb")
                    nc.scalar.copy(ss_sb[:, 0:w], ss_ps[:, 0:w])
                    nc.sync.dma_start(
                        out=cck_in[moff : moff + w].rearrange(
                            "(a n) -> a n", a=1
                        ),
                        in_=ss_sb[:, 0:w],
                    )
                    # v from the same resident block
                    for ct in range(w // 128):
                        mtg = moff // 128 + ct
                        psv = vps.tile([128, 256], f32, tag="v")
                        for et in range(8):
                            nc.tensor.matmul(
                                psv,
                                blk[:, et, ct * 128 : ct * 128 + 128],
                                wv_sb[:, et, :],
                                start=(et == 0),
                                stop=(et == 7),
                            )
                        nc.vector.tensor_copy(
                            v_sb[:, mtg, :, 0:C],
                            psv.rearrange("p (h c) -> p h c", c=C),
                        )

                nc.gpsimd.collective_compute(
                    "AllReduce",
                    mybir.AluOpType.add,
                    replica_groups=[[0, 1, 2, 3], [4, 5, 6, 7]],
                    ins=[cck_in.opt()],
                    outs=[cck_out.opt()],
                )

                # ---- q projection: qT[dt, nb] = Wq'^T-slice @ q-block ----
                for nb in range(4):
                    blk = blkpool.tile([128, 8, 512], f32r, tag="blk")
                    for et in range(8):
                        nc.sync.dma_start(
                            out=blk[:, et, :],
                            in_=qT_d[et * 128 : et * 128 + 128, nb * 512 : nb * 512 + 512],
                        )
                    ss_ps = ssps.tile([1, 512], f32, tag="ss")
                    for dt in range(2):
                        ps = projps.tile([128, 512], f32, tag="proj")
                        for et in range(8):
                            nc.tensor.matmul(
                                ps,
                                wq_sb[:, et, dt * 128 : dt * 128 + 128],
                                blk[:, et, :],
                                start=(et == 0),
                                stop=(et == 7),
                            )
                        nc.vector.tensor_copy(qT[:, dt, nb, :], ps)
                        sq = sqpool.tile([128, 512], f32r, tag="sq")
                        nc.vector.tensor_mul(sq, qT[:, dt, nb, :], qT[:, dt, nb, :])
                        nc.tensor.matmul(
                            ss_ps,
                            ig2q_sb[:, dt : dt + 1],
                            sq,
                            start=(dt == 0),
                            stop=(dt == 1),
                            skip_group_check=True,
                        )
                    ss_sb = small.tile([1, 512], f32, tag="ss_sb")
                    nc.scalar.copy(ss_sb, ss_ps)
                    nc.sync.dma_start(
                        out=ccq_in[nb * 512 : nb * 512 + 512].rearrange(
                            "(a n) -> a n", a=1
                        ),
                        in_=ss_sb,
                    )

                # AR_q fires while the kv projection runs; AR_k pipelines
                # behind it on the CC queue.
                nc.gpsimd.collective_compute(
                    "AllReduce",
                    mybir.AluOpType.add,
                    replica_groups=[[0, 1, 2, 3], [4, 5, 6, 7]],
                    ins=[ccq_in.opt()],
                    outs=[ccq_out.opt()],
                )

                # wo weight load (DMA only; hides under the AllReduce)
                wo_sb = wts.tile([128, 2, D], f32r, tag="w")
                for dc in range(2):
                    nc.sync.dma_start(
                        out=wo_sb[:, dc, :], in_=woT_d[dc * 128 : dc * 128 + 128, :]
                    )

                # ---- warm burst: dependency-free matmuls spanning the
                # AllReduce latency so the PE HAM clock-gate stays at 8/8
                # when attention starts (a PE-idle window here would drop it
                # to 4/8 and the attention stream would start cold) ----
                warm = projps.tile([128, 512], f32, tag="proj")
                for i in range(WARM_MM):
                    nc.tensor.matmul(
                        warm,
                        kT[:, 0, 0:128],
                        kT[:, 0, 512:1024],
                        start=(i == 0),
                        stop=(i == WARM_MM - 1),
                        skip_group_check=True,
                    )
                warm_sink = small.tile([1, 512], f32, tag="rd")
                nc.vector.tensor_copy(warm_sink, warm[0:1, :])

                # ---- rstd_k: [128, n_mt] lane-parallel; feeds exp scale ----
                ss128 = small.tile([128, n_mt], f32, tag="ssk")
                nc.sync.dma_start(
                    out=ss128, in_=cck_out.rearrange("(t p) -> p t", p=128)
                )
                stdk = small.tile([128, n_mt], f32, tag="stdk")
                nc.scalar.activation(stdk, ss128, AF.Sqrt, bias=eps_t, scale=invd_t)
                rstdk = singles.tile([128, n_mt], f32)
                nc.vector.reciprocal_approx_fast(out=rstdk, in_=stdk)

                # ---- rstd_q: lane-parallel [128, 16], then a DRAM bounce to
                # the [1, 2048] row layout the bcast outer-product wants
                # (single-lane sqrt/recip on [1, 2048] costs ~5us; this ~1us)
                ssq128 = small.tile([128, 16], f32, tag="ssq")
                nc.sync.dma_start(
                    out=ssq128, in_=ccq_out.rearrange("(t p) -> p t", p=128)
                )
                stdq = small.tile([128, 16], f32, tag="stdq")
                nc.scalar.activation(stdq, ssq128, AF.Sqrt, bias=eps_t, scale=invd_t)
                rsq128 = small.tile([128, 16], f32, tag="rsq")
                nc.vector.reciprocal_approx_fast(out=rsq128, in_=stdq)
                nc.sync.dma_start(
                    out=rsq_dram.rearrange("(t p) -> p t", p=128), in_=rsq128
                )
                rs_row = singles.tile([1, 2048], f32)
                nc.sync.dma_start(
                    out=rs_row, in_=rsq_dram.rearrange("(a n) -> a n", a=1)
                )
                # q finalize: qT[d, n] *= rstd_q[n] via ones outer-product bcast
                for nb in range(4):
                    bcq = projps.tile([128, 512], f32, tag="proj")
                    nc.tensor.matmul(
                        bcq,
                        ones1x128,
                        rs_row[:, nb * 512 : nb * 512 + 512],
                        start=True,
                        stop=True,
                    )
                    for dt in range(2):
                        nc.vector.tensor_mul(qT[:, dt, nb, :], qT[:, dt, nb, :], bcq)

            # ---- attention: flat stream over g = (pass, mt), pipelined
            # depth-2 ACROSS pass boundaries.  PE order:
            #   ... PV(g-1), QK(g+1), filler, PV(g), QK(g+2), ...
            # ACT order: exp(0), exp(1), ...  The PE never waits on an exp
            # (QK(g+1)'s s2 slot was freed by exp(g-1), one full exp ago) so
            # the HAM clock-gate stays warm.
            # PSUM banks: s2 2x2 + o2 2 + filler/bc/dummy 2x1 = 8.
            with (
                tc.tile_pool(name="sps", bufs=2, space="PSUM") as spool,
                tc.tile_pool(name="ops", bufs=1, space="PSUM") as opool,
                tc.tile_pool(name="fps", bufs=2, space="PSUM") as fpool,
            ):
                passes = [(nbp, h) for nbp in range(2) for h in range(4)]
                G = len(passes) * n_mt
                s2g, p2g, o2cur = {}, {}, {}
                filler = []
                prev = [None]

                def emit_dummy():
                    dum = fpool.tile([128, 512], f32, tag="f")
                    nc.tensor.matmul(
                        dum, kT[:, 0, 0:128], kT[:, 0, 512:1024],
                        start=True, stop=True, skip_group_check=True,
                    )

                def make_outproj(nbp):
                    """one closure per (nb, ot) psum tile: 2 matmuls + copy + dma"""
                    fns = []
                    for nb in (2 * nbp, 2 * nbp + 1):
                        for ot in range(8):
                            def f(nb=nb, ot=ot):
                                ps = fpool.tile([128, 512], f32, tag="f")
                                for dc in range(2):
                                    nc.tensor.matmul(
                                        ps,
                                        wo_sb[:, dc, ot * 128 : ot * 128 + 128],
                                        xT[:, dc, nb, :],
                                        start=(dc == 0),
                                        stop=(dc == 1),
                                    )
                                out_sb = outbuf.tile([128, 512], f32, tag="osb")
                                nc.vector.tensor_copy(out_sb, ps)
                                nc.sync.dma_start(
                                    out=outT_d[
                                        ot * 128 : ot * 128 + 128,
                                        nb * 512 : nb * 512 + 512,
                                    ],
                                    in_=out_sb,
                                )
                            fns.append(f)
                    return fns

                def emit_normalize(state):
                    """bc outer-products + muls for a pass whose DVE recips are
                    done by now (emitted one pass late to keep PE gapless)."""
                    hh, nbp_, oo_sb, rds_ = state
                    ddt, ooff = hh // 2, (hh % 2) * C
                    for i, nb in enumerate((2 * nbp_, 2 * nbp_ + 1)):
                        bc = fpool.tile([128, 512], f32, tag="f")
                        nc.tensor.matmul(
                            bc[0:C, :], ones1x64, rds_[i], start=True, stop=True
                        )
                        nc.vector.tensor_mul(
                            xT[ooff : ooff + C, ddt, nb, :],
                            oo_sb[0:C, i, :],
                            bc[0:C, :],
                        )

                def emit_qk(g):
                    (nbp, h), mt = passes[g // n_mt], g % n_mt
                    dt, off = h // 2, (h % 2) * C
                    s2 = spool.tile([128, 2, 512], f32, tag="s2")
                    kT_lhs = kT[off : off + C, dt, mt * 128 : mt * 128 + 128]
                    for i, nb in enumerate((2 * nbp, 2 * nbp + 1)):
                        nc.tensor.matmul(
                            s2[:, i, :],
                            kT_lhs,
                            qT[off : off + C, dt, nb, :],
                            start=True,
                            stop=True,
                        )
                    s2g[g] = s2

                def emit_exp(g):
                    mt = g % n_mt
                    p2 = ppool.tile([128, 2, 512], bf16, tag="p")
                    nc.scalar.activation(
                        p2, s2g.pop(g), AF.Exp,
                        bias=mb_sb[:, mt : mt + 1],
                        scale=rstdk[:, mt : mt + 1],
                    )
                    p2g[g] = p2

                def emit_pv(g):
                    pi, mt = g // n_mt, g % n_mt
                    nbp, h = passes[pi]
                    if mt == 0:
                        o2_t = opool.tile([C + 1, 2, 512], f32, tag="o2")
                        o2cur[pi] = o2_t
                    o2 = o2cur[pi]
                    p2 = p2g.pop(g)
                    for i in range(2):
                        nc.tensor.matmul(
                            o2[:, i, :],
                            v_sb[:, mt, h, :],
                            p2[:, i, :],
                            start=(mt == 0),
                            stop=(mt == n_mt - 1),
                            skip_group_check=True,
                        )
                    if mt == n_mt - 1:
                        # pass drain: free o2 with one DVE copy; recips; the
                        # PREVIOUS pass's normalize (its recips are done)
                        o_sb = obuf.tile([C + 1, 2, 512], f32, tag="osb")
                        nc.vector.tensor_copy(o_sb, o2cur.pop(pi))
                        rds = []
                        for i in range(2):
                            den = rdp.tile([1, 512], f32, tag="den")
                            nc.vector.tensor_copy(den, o_sb[C : C + 1, i, :])
                            rd = rdp.tile([1, 512], f32, tag="rd")
                            nc.vector.reciprocal_approx_fast(out=rd, in_=den)
                            rds.append(rd)
                        if prev[0] is not None:
                            emit_normalize(prev[0])
                            if prev[0][1] == 0 and prev[0][0] == 3:
                                # nbp=0 fully normalized -> its out-projection
                                # becomes PE filler for the remaining passes
                                filler.extend(make_outproj(0))
                        prev[0] = (h, nbp, o_sb, rds)

                emit_qk(0)
                emit_qk(1)
                rewarm = fpool.tile([128, 512], f32, tag="f")
                for i in range(40):
                    nc.tensor.matmul(
                        rewarm, kT[:, 0, 0:128], kT[:, 0, 512:1024],
                        start=(i == 0), stop=(i == 39), skip_group_check=True,
                    )
                emit_exp(0)
                for _ in range(2):
                    emit_dummy()
                for g in range(1, G):
                    emit_pv(g - 1)
                    if g + 1 < G:
                        emit_qk(g + 1)
                    if filler:
                        filler.pop(0)()
                    else:
                        emit_dummy()
                    emit_exp(g)
                emit_pv(G - 1)

                emit_normalize(prev[0])
                # tail: whatever filler wasn't consumed + second-half out proj
                for f in filler:
                    f()
                for f in make_outproj(1):
                    f()

    nc.finalize()
    return nc


_NC_CACHE = {}


def _get_nc(n_mt: int):
    if n_mt not in _NC_CACHE:
        _NC_CACHE[n_mt] = build(n_mt)
    return _NC_CACHE[n_mt]


def n_mt_for(mask) -> int:
    mask = np.asarray(mask)
    cnt = int(max((mask[b] != 0).sum() for b in range(B)))
    return max(1, (cnt + 127) // 128)


def make_in_maps(querys, key_feats, mask, Wq, Wk, Wv, gq, gk, Wo, bo, n_mt):
    M = n_mt * 128
    querys = np.asarray(querys, dtype=np.float32)
    key_feats = np.asarray(key_feats, dtype=np.float32)
    mask = np.asarray(mask)
    gq = np.asarray(gq, dtype=np.float32)
    gk = np.asarray(gk, dtype=np.float32)

    gsq_full = gq * np.float32(SCALE)  # folded into Wq rows
    gsk_full = gk.astype(np.float32)  # folded into Wk rows
    Wq_f = np.asarray(Wq, dtype=np.float32) * gsq_full[:, None]
    Wk_f = np.asarray(Wk, dtype=np.float32) * gsk_full[:, None]

    qT = [round_f32r(querys[b].T) for b in range(B)]
    kfT, mb = [], []
    for b in range(B):
        idx = np.flatnonzero(mask[b] != 0)
        cnt = len(idx)
        kc = np.zeros((M, E), np.float32)
        kc[:cnt] = key_feats[b][idx]
        kfT.append(round_f32r(kc.T))
        mbv = np.full((M,), NEG, dtype=np.float32)
        mbv[:cnt] = 0.0
        mb.append(mbv.reshape(n_mt, 128))

    wqT, wkT, wvT, woT, ig2q, ig2k = [], [], [], [], [], []
    for j in range(4):
        dsl = slice(j * DS, (j + 1) * DS)
        wqT.append(round_f32r(Wq_f[dsl].T))
        wkT.append(round_f32r(Wk_f[dsl].T))
        wvT.append(round_f32r(np.asarray(Wv)[dsl].T))
        woT.append(round_f32r(np.asarray(Wo)[:, dsl].T))
        # sumsq compensation: raw sumsq = sum_d (q'_d)^2 / gs_d^2
        ig2q.append(round_f32r((1.0 / gsq_full[dsl] ** 2).reshape(2, 128)))
        ig2k.append(round_f32r((1.0 / gsk_full[dsl] ** 2).reshape(2, 128)))

    in_maps = []
    for cid in range(NCORES):
        b, j = cid // 4, cid % 4
        in_maps.append(
            {
                "qT": qT[b],
                "kfT": kfT[b],
                "wqT": wqT[j],
                "wkT": wkT[j],
                "wvT": wvT[j],
                "woT": woT[j],
                "ig2q": ig2q[j],
                "ig2k": ig2k[j],
                "mbias": mb[b],
            }
        )
    return in_maps


def assemble(results, bo):
    bo = np.asarray(bo, dtype=np.float32)
    out = np.zeros((B, N, D), dtype=np.float32)
    for cid in range(NCORES):
        b = cid // 4
        out[b] += results[cid]["outT"].T
    out += bo
    return out


def kernel(querys, key_feats, mask, Wq, Wk, Wv, gq, gk, Wo, bo):
    n_mt = n_mt_for(mask)
    nc = _get_nc(n_mt)
    in_maps = make_in_maps(querys, key_feats, mask, Wq, Wk, Wv, gq, gk, Wo, bo, n_mt)
    res = run_bass_kernel_spmd(nc, in_maps, list(range(NCORES)))
    return assemble(res.results, bo)


# revision 19
# speedup vs baseline: 1.1864x; 1.0954x over previous
"""CrossAttention Trainium2 kernel (8 NeuronCores).

Reference computation (B=2, N=M=2048, D=1024, H=16, C=64):
    q = rmsnorm(querys @ Wq.T, gq) * C**-0.5       [B,N,D]
    k = rmsnorm(key_feats @ Wk.T, gk)              [B,M,D]
    v = key_feats @ Wv.T                           [B,M,D]
    attn = softmax(mask(q @ k.T per head))         [B,H,N,M]
    out = (attn @ v per head, concat) @ Wo.T + bo  [B,N,D]

Sharding: core = b*4 + j (b in {0,1}; j in {0..3} owns heads 4j..4j+3 = a
256-wide slice of D). Host pre-transposes inputs/weights, folds gq*scale /
gk into Wq / Wk rows, and pre-rounds everything to f32r (fp32 with 11-bit
mantissa -> full PE rate).

v2 structural changes vs v1:
  - Mask compaction: rows with mask==0 contribute exp(-inf)=0 to both the
    softmax denominator and PV, so the host gathers only the valid kf
    columns (per batch), pads to a multiple of 128, and the kernel runs
    with M_pad ~= 1152 instead of 2048.  All M-side work (k/v projection,
    QK, exp, PV, kf DMA) shrinks ~2x.  Padding columns carry bias -1e30
    into the exp -> contribute exactly 0.
  - k and v projections fused over a single kfT stream (halves kf DMA).
  - One fused AllReduce carries both q and k partial sum-of-squares
    (2048 + M_pad floats); its ~27us mesh latency is bridged by a long
    dependency-free dummy-matmul burst that keeps the PE HAM clock-gate
    warm (K=8/8) into attention.
  - Attention is software-pipelined depth-2 per (nbp, h) pass: PE order is
    ... PV(mt-1), QK(mt+1), filler, PV(mt) ... so the PE never stalls on
    the ACT exp (v1 stalled ~0.4us every mt, which kept HAM at K=4/8 =
    1.2 GHz for the whole 314us attention phase).  ACT exp (~1.2us/mt) is
    the pace-setter; PE real work is ~1.0us/mt, padded by a dummy matmul
    (first n-half) or an out-projection matmul pair (second n-half).
  - The out projection (partial over this core's d-slice) is interleaved
    into attention as filler work; the host sums 4 partials per b and
    adds bo.
"""

import numpy as np

import concourse.tile as tile
from concourse import bacc, mybir
from concourse.bass_utils import run_bass_kernel_spmd

B, N, M_FULL, D, H = 2, 2048, 2048, 1024, 16
C = D // H  # 64, head dim
E = D  # input feature dim
EPS = 1e-6
SCALE = C ** (-0.5)
DS = D // 4  # 256, per-core d-slice
NCORES = 8

f32 = mybir.dt.float32
f32r = mybir.dt.float32r
bf16 = mybir.dt.bfloat16
AF = mybir.ActivationFunctionType

NEG = -1e30
WARM_MM = 60  # dummy matmuls bridging the AllReduce latency into attention


def round_f32r(x: np.ndarray) -> np.ndarray:
    b = np.ascontiguousarray(x, dtype=np.float32).view(np.uint32)
    b = (b + 0x800) & np.uint32(0xFFFFF000)
    return b.view(np.float32)


def build(n_mt: int):
    M = n_mt * 128
    mblocks = []
    off = 0
    while off < M:
        w = min(512, M - off)
        mblocks.append((off, w))
        off += w

    nc = bacc.Bacc(None, target_bir_lowering=False)

    qT_d = nc.declare_dram_parameter("qT", [E, N], f32r, isOutput=False)
    kfT_d = nc.declare_dram_parameter("kfT", [E, M], f32r, isOutput=False)
    wqT_d = nc.declare_dram_parameter("wqT", [E, DS], f32r, isOutput=False)
    wkT_d = nc.declare_dram_parameter("wkT", [E, DS], f32r, isOutput=False)
    wvT_d = nc.declare_dram_parameter("wvT", [E, DS], f32r, isOutput=False)
    woT_d = nc.declare_dram_parameter("woT", [DS, D], f32r, isOutput=False)
    ig2q_d = nc.declare_dram_parameter("ig2q", [2, 128], f32r, isOutput=False)
    ig2k_d = nc.declare_dram_parameter("ig2k", [2, 128], f32r, isOutput=False)
    mb_d = nc.declare_dram_parameter("mbias", [n_mt, 128], f32, isOutput=False)
    outT_d = nc.declare_dram_parameter("outT", [D, N], f32, isOutput=True)

    with (
        nc.allow_low_precision(reason="f32r matmul operands by design; fp32 PSUM"),
        tile.TileContext(nc) as tc,
    ):
        with (
            tc.tile_pool(name="singles", bufs=1) as singles,
            tc.tile_pool(name="wts", bufs=3) as wts,
            tc.tile_pool(name="blk", bufs=2) as blkpool,
            tc.tile_pool(name="sq", bufs=2) as sqpool,
            tc.tile_pool(name="psb", bufs=3) as ppool,
            tc.tile_pool(name="obuf", bufs=2) as obuf,
            tc.tile_pool(name="osb2", bufs=10) as outbuf,
            tc.tile_pool(name="rdp", bufs=4) as rdp,
            tc.tile_pool(name="small", bufs=2) as small,
            tc.tile_pool(name="dram", bufs=1, space="DRAM") as dram,
        ):
            # ---- constants / small inputs ----
            ones1x64 = singles.tile([1, 64], f32)
            nc.vector.memset(ones1x64, 1.0)
            ones1x128 = singles.tile([1, 128], f32)
            nc.vector.memset(ones1x128, 1.0)
            onesv = singles.tile([128, n_mt * 4], f32)
            nc.vector.memset(onesv, 1.0)
            eps_t = singles.tile([128, 1], f32)
            nc.vector.memset(eps_t, EPS)
            invd_t = singles.tile([128, 1], f32)
            nc.vector.memset(invd_t, 1.0 / D)
            ig2q_sb = singles.tile([128, 2], f32r)
            nc.sync.dma_start(out=ig2q_sb, in_=ig2q_d.rearrange("t p -> p t"))
            ig2k_sb = singles.tile([128, 2], f32r)
            nc.sync.dma_start(out=ig2k_sb, in_=ig2k_d.rearrange("t p -> p t"))
            mb_sb = singles.tile([128, n_mt], f32)
            nc.sync.dma_start(out=mb_sb, in_=mb_d.rearrange("t p -> p t"))

            # weights: wq, wk, wv upfront; wo reuses wq's slot after q proj
            wq_sb = wts.tile([128, 8, DS], f32r, tag="w")
            wk_sb = wts.tile([128, 8, DS], f32r, tag="w")
            wv_sb = wts.tile([128, 8, DS], f32r, tag="w")
            for et in range(8):
                nc.sync.dma_start(out=wq_sb[:, et, :], in_=wqT_d[et * 128 : et * 128 + 128, :])
                nc.scalar.dma_start(out=wk_sb[:, et, :], in_=wkT_d[et * 128 : et * 128 + 128, :])
                nc.scalar.dma_start(out=wv_sb[:, et, :], in_=wvT_d[et * 128 : et * 128 + 128, :])

            # ---- persistent activations ----
            qT = singles.tile([128, 2, 4, 512], f32r)  # [p, dt, nb, n]
            kT = singles.tile([128, 2, M], f32r)  # [p, dt, m]
            v_sb = singles.tile([128, n_mt, 4, C + 1], bf16)  # [m_p, mt, h, c|ones]
            xT = singles.tile([128, 2, 4, 512], f32r)  # [p, dt, nb, n]
            nc.vector.tensor_copy(
                v_sb[:, :, :, C], onesv.rearrange("p (a b) -> p a b", a=n_mt)
            )

            ccq_in = dram.tile([2048], f32)
            ccq_out = dram.tile([2048], f32)
            cck_in = dram.tile([M], f32)
            cck_out = dram.tile([M], f32)
            rsq_dram = dram.tile([2048], f32)

            with (
                tc.tile_pool(name="projps", bufs=2, space="PSUM") as projps,
                tc.tile_pool(name="vps", bufs=2, space="PSUM") as vps,
                tc.tile_pool(name="ssps", bufs=2, space="PSUM") as ssps,
            ):
                # ---- fused k+v projection over one kfT stream ----
                for moff, w in mblocks:
                    blk = blkpool.tile([128, 8, 512], f32r, tag="blk")
                    for et in range(8):
                        nc.scalar.dma_start(
                            out=blk[:, et, 0:w],
                            in_=kfT_d[et * 128 : et * 128 + 128, moff : moff + w],
                        )
                    ss_ps = ssps.tile([1, 512], f32, tag="ss")
                    for dt in range(2):
                        ps = projps.tile([128, 512], f32, tag="proj")
                        for et in range(8):
                            nc.tensor.matmul(
                                ps[:, 0:w],
                                wk_sb[:, et, dt * 128 : dt * 128 + 128],
                                blk[:, et, 0:w],
                                start=(et == 0),
                                stop=(et == 7),
                            )
                        nc.vector.tensor_copy(kT[:, dt, moff : moff + w], ps[:, 0:w])
                        sq = sqpool.tile([128, 512], f32r, tag="sq")
                        nc.vector.tensor_mul(
                            sq[:, 0:w],
                            kT[:, dt, moff : moff + w],
                            kT[:, dt, moff : moff + w],
                        )
                        nc.tensor.matmul(
                            ss_ps[:, 0:w],
                            ig2k_sb[:, dt : dt + 1],
                            sq[:, 0:w],
                            start=(dt == 0),
                            stop=(dt == 1),
                            skip_group_check=True,
                        )
                    ss_sb = small.tile([1, 512], f32, tag="ss_sb")
                    nc.scalar.copy(ss_sb[:, 0:w], ss_ps[:, 0:w])
                    nc.sync.dma_start(
                        out=cck_in[moff : moff + w].rearrange(
                            "(a n) -> a n", a=1
                        ),
                        in_=ss_sb[:, 0:w],
                    )
                    # v from the same resident block
                    for ct in range(w // 128):
                        mtg = moff // 128 + ct
                        psv = vps.tile([128, 256], f32, tag="v")
                        for et in range(8):
                            nc.tensor.matmul(
                                psv,
                                blk[:, et, ct * 128 : ct * 128 + 128],
                                wv_sb[:, et, :],
                                start=(et == 0),
                                stop=(et == 7),
                            )
                        nc.vector.tensor_copy(
                            v_sb[:, mtg, :, 0:C],
                            psv.rearrange("p (h c) -> p h c", c=C),
                        )

                nc.gpsimd.collective_compute(
                    "AllReduce",
                    mybir.AluOpType.add,
                    replica_groups=[[0, 1, 2, 3], [4, 5, 6, 7]],
                    ins=[cck_in.opt()],
                    outs=[cck_out.opt()],
                )

                # ---- q projection: qT[dt, nb] = Wq'^T-slice @ q-block ----
                for nb in range(4):
                    blk = blkpool.tile([128, 8, 512], f32r, tag="blk")
                    for et in range(8):
                        nc.sync.dma_start(
                            out=blk[:, et, :],
                            in_=qT_d[et * 128 : et * 128 + 128, nb * 512 : nb * 512 + 512],
                        )
                    ss_ps = ssps.tile([1, 512], f32, tag="ss")
                    for dt in range(2):
                        ps = projps.tile([128, 512], f32, tag="proj")
                        for et in range(8):
                            nc.tensor.matmul(
                                ps,
                                wq_sb[:, et, dt * 128 : dt * 128 + 128],
                                blk[:, et, :],
                                start=(et == 0),
                                stop=(et == 7),
                            )
                        nc.vector.tensor_copy(qT[:, dt, nb, :], ps)
                        sq = sqpool.tile([128, 512], f32r, tag="sq")
                        nc.vector.tensor_mul(sq, qT[:, dt, nb, :], qT[:, dt, nb, :])
                        nc.tensor.matmul(
                            ss_ps,
                            ig2q_sb[:, dt : dt + 1],
                            sq,
                            start=(dt == 0),
                            stop=(dt == 1),
                            skip_group_check=True,
                        )
                    ss_sb = small.tile([1, 512], f32, tag="ss_sb")
                    nc.scalar.copy(ss_sb, ss_ps)
                    nc.sync.dma_start(
                        out=ccq_in[nb * 512 : nb * 512 + 512].rearrange(
                            "(a n) -> a n", a=1
                        ),
                        in_=ss_sb,
                    )

                # AR_q fires while the kv projection runs; AR_k pipelines
                # behind it on the CC queue.
                nc.gpsimd.collective_compute(
                    "AllReduce",
                    mybir.AluOpType.add,
                    replica_groups=[[0, 1, 2, 3], [4, 5, 6, 7]],
                    ins=[ccq_in.opt()],
                    outs=[ccq_out.opt()],
                )

                # wo weight load (DMA only; hides under the AllReduce)
                wo_sb = wts.tile([128, 2, D], f32r, tag="w")
                for dc in range(2):
                    nc.sync.dma_start(
                        out=wo_sb[:, dc, :], in_=woT_d[dc * 128 : dc * 128 + 128, :]
                    )

                # ---- warm burst: dependency-free matmuls spanning the
                # AllReduce latency so the PE HAM clock-gate stays at 8/8
                # when attention starts (a PE-idle window here would drop it
                # to 4/8 and the attention stream would start cold) ----
                warm = projps.tile([128, 512], f32, tag="proj")
                for i in range(WARM_MM):
                    nc.tensor.matmul(
                        warm,
                        kT[:, 0, 0:128],
                        kT[:, 0, 512:1024],
                        start=(i == 0),
                        stop=(i == WARM_MM - 1),
                        skip_group_check=True,
                    )
                warm_sink = small.tile([1, 512], f32, tag="rd")
                nc.vector.tensor_copy(warm_sink, warm[0:1, :])

                # ---- rstd_k: [128, n_mt] lane-parallel; feeds exp scale ----
                ss128 = small.tile([128, n_mt], f32, tag="ssk")
                nc.sync.dma_start(
                    out=ss128, in_=cck_out.rearrange("(t p) -> p t", p=128)
                )
                stdk = small.tile([128, n_mt], f32, tag="stdk")
                nc.scalar.activation(stdk, ss128, AF.Sqrt, bias=eps_t, scale=invd_t)
                rstdk = singles.tile([128, n_mt], f32)
                nc.vector.reciprocal_approx_fast(out=rstdk, in_=stdk)

                # ---- rstd_q: lane-parallel [128, 16], then a DRAM bounce to
                # the [1, 2048] row layout the bcast outer-product wants
                # (single-lane sqrt/recip on [1, 2048] costs ~5us; this ~1us)
                ssq128 = small.tile([128, 16], f32, tag="ssq")
                nc.sync.dma_start(
                    out=ssq128, in_=ccq_out.rearrange("(t p) -> p t", p=128)
                )
                stdq = small.tile([128, 16], f32, tag="stdq")
                nc.scalar.activation(stdq, ssq128, AF.Sqrt, bias=eps_t, scale=invd_t)
                rsq128 = small.tile([128, 16], f32, tag="rsq")
                nc.vector.reciprocal_approx_fast(out=rsq128, in_=stdq)
                nc.sync.dma_start(
                    out=rsq_dram.rearrange("(t p) -> p t", p=128), in_=rsq128
                )
                rs_row = singles.tile([1, 2048], f32)
                nc.sync.dma_start(
                    out=rs_row, in_=rsq_dram.rearrange("(a n) -> a n", a=1)
                )
                # q finalize: qT[d, n] *= rstd_q[n] via ones outer-product bcast
                for nb in range(4):
                    bcq = projps.tile([128, 512], f32, tag="proj")
                    nc.tensor.matmul(
                        bcq,
                        ones1x128,
                        rs_row[:, nb * 512 : nb * 512 + 512],
                        start=True,
                        stop=True,
                    )
                    for dt in range(2):
                        nc.vector.tensor_mul(qT[:, dt, nb, :], qT[:, dt, nb, :], bcq)

            # ---- attention: flat stream over g = (pass, mt), pipelined
            # depth-2 ACROSS pass boundaries.  PE order:
            #   ... PV(g-1), QK(g+1), filler, PV(g), QK(g+2), ...
            # ACT order: exp(0), exp(1), ...  The PE never waits on an exp
            # (QK(g+1)'s s2 slot was freed by exp(g-1), one full exp ago) so
            # the HAM clock-gate stays warm.
            # PSUM banks: s2 2x2 + o2 2 + filler/bc/dummy 2x1 = 8.
            with (
                tc.tile_pool(name="sps", bufs=2, space="PSUM") as spool,
                tc.tile_pool(name="ops", bufs=1, space="PSUM") as opool,
                tc.tile_pool(name="fps", bufs=2, space="PSUM") as fpool,
            ):
                # passes are (head-pair hp, nb): the two heads of a pair sit
                # at partitions 0:64 / 64:128 of dt=hp, so their QK matmuls
                # run CONCURRENTLY as row-group tiles (K=64 each) -- one
                # 512-cycle stream produces both heads' s2.  nb-major order
                # so each nb's out-projection unlocks early as PE filler.
                passes = [(hp, nb) for nb in range(4) for hp in range(2)]
                G = len(passes) * n_mt
                s2g, p2g, o2cur = {}, {}, {}
                filler = []
                prev = [None]

                def emit_dummy():
                    dum = fpool.tile([128, 512], f32, tag="f")
                    nc.tensor.matmul(
                        dum, kT[:, 0, 0:128], kT[:, 0, 512:1024],
                        start=True, stop=True, skip_group_check=True,
                    )

                def make_outproj(nb):
                    """one closure per ot psum tile: 2 matmuls + copy + dma"""
                    fns = []
                    for ot in range(8):
                        def f(nb=nb, ot=ot):
                            ps = fpool.tile([128, 512], f32, tag="f")
                            for dc in range(2):
                                nc.tensor.matmul(
                                    ps,
                                    wo_sb[:, dc, ot * 128 : ot * 128 + 128],
                                    xT[:, dc, nb, :],
                                    start=(dc == 0),
                                    stop=(dc == 1),
                                )
                            out_sb = outbuf.tile([128, 512], f32, tag="osb")
                            nc.vector.tensor_copy(out_sb, ps)
                            nc.sync.dma_start(
                                out=outT_d[
                                    ot * 128 : ot * 128 + 128,
                                    nb * 512 : nb * 512 + 512,
                                ],
                                in_=out_sb,
                            )
                        fns.append(f)
                    return fns

                def emit_normalize(state):
                    """bc outer-products + muls for a pass whose DVE recips are
                    done by now (emitted one pass late to keep PE gapless)."""
                    hp_, nb_, oo_sb, rds_ = state
                    for i in range(2):
                        bc = fpool.tile([128, 512], f32, tag="f")
                        nc.tensor.matmul(
                            bc[0:C, :], ones1x64, rds_[i], start=True, stop=True
                        )
                        nc.vector.tensor_mul(
                            xT[i * 64 : i * 64 + C, hp_, nb_, :],
                            oo_sb[0:C, i, :],
                            bc[0:C, :],
                        )

                def emit_qk(g):
                    (hp, nb), mt = passes[g // n_mt], g % n_mt
                    s2 = spool.tile([128, 2, 512], f32, tag="s2")
                    for i in range(2):
                        nc.tensor.matmul(
                            s2[:, i, :],
                            kT[i * 64 : i * 64 + 64, hp, mt * 128 : mt * 128 + 128],
                            qT[i * 64 : i * 64 + 64, hp, nb, :],
                            start=True,
                            stop=True,
                        )
                    s2g[g] = s2

                def emit_exp(g):
                    mt = g % n_mt
                    p2 = ppool.tile([128, 2, 512], bf16, tag="p")
                    nc.scalar.activation(
                        p2, s2g.pop(g), AF.Exp,
                        bias=mb_sb[:, mt : mt + 1],
                        scale=rstdk[:, mt : mt + 1],
                    )
                    p2g[g] = p2

                def emit_pv(g):
                    pi, mt = g // n_mt, g % n_mt
                    hp, nb = passes[pi]
                    if mt == 0:
                        o2_t = opool.tile([C + 1, 2, 512], f32, tag="o2")
                        o2cur[pi] = o2_t
                    o2 = o2cur[pi]
                    p2 = p2g.pop(g)
                    for i in range(2):
                        nc.tensor.matmul(
                            o2[:, i, :],
                            v_sb[:, mt, 2 * hp + i, :],
                            p2[:, i, :],
                            start=(mt == 0),
                            stop=(mt == n_mt - 1),
                            skip_group_check=True,
                        )
                    if mt == n_mt - 1:
                        # pass drain: free o2 with one DVE copy; recips; the
                        # PREVIOUS pass's normalize (its recips are done)
                        o_sb = obuf.tile([C + 1, 2, 512], f32, tag="osb")
                        nc.vector.tensor_copy(o_sb, o2cur.pop(pi))
                        rds = []
                        for i in range(2):
                            den = rdp.tile([1, 512], f32, tag="den")
                            nc.vector.tensor_copy(den, o_sb[C : C + 1, i, :])
                            rd = rdp.tile([1, 512], f32, tag="rd")
                            nc.vector.reciprocal_approx_fast(out=rd, in_=den)
                            rds.append(rd)
                        if prev[0] is not None:
                            emit_normalize(prev[0])
                            if prev[0][0] == 1 and prev[0][1] < 3:
                                # both head-pairs of nb done -> its out
                                # projection becomes PE filler
                                filler.extend(make_outproj(prev[0][1]))
                        prev[0] = (hp, nb, o_sb, rds)

                emit_qk(0)
                emit_qk(1)
                rewarm = fpool.tile([128, 512], f32, tag="f")
                for i in range(40):
                    nc.tensor.matmul(
                        rewarm, kT[:, 0, 0:128], kT[:, 0, 512:1024],
                        start=(i == 0), stop=(i == 39), skip_group_check=True,
                    )
                emit_exp(0)
                for _ in range(2):
                    emit_dummy()
                for g in range(1, G):
                    emit_pv(g - 1)
                    if g + 1 < G:
                        emit_qk(g + 1)
                    if filler:
                        filler.pop(0)()
                    else:
                        emit_dummy()
                        emit_dummy()
                    emit_exp(g)
                emit_pv(G - 1)

                emit_normalize(prev[0])
                # tail: whatever filler wasn't consumed + last nb's out proj
                for f in filler:
                    f()
                for f in make_outproj(3):
                    f()

    nc.finalize()
    return nc


_NC_CACHE = {}


def _get_nc(n_mt: int):
    if n_mt not in _NC_CACHE:
        _NC_CACHE[n_mt] = build(n_mt)
    return _NC_CACHE[n_mt]


def n_mt_for(mask) -> int:
    mask = np.asarray(mask)
    cnt = int(max((mask[b] != 0).sum() for b in range(B)))
    return max(1, (cnt + 127) // 128)


def make_in_maps(querys, key_feats, mask, Wq, Wk, Wv, gq, gk, Wo, bo, n_mt):
    M = n_mt * 128
    querys = np.asarray(querys, dtype=np.float32)
    key_feats = np.asarray(key_feats, dtype=np.float32)
    mask = np.asarray(mask)
    gq = np.asarray(gq, dtype=np.float32)
    gk = np.asarray(gk, dtype=np.float32)

    gsq_full = gq * np.float32(SCALE)  # folded into Wq rows
    gsk_full = gk.astype(np.float32)  # folded into Wk rows
    Wq_f = np.asarray(Wq, dtype=np.float32) * gsq_full[:, None]
    Wk_f = np.asarray(Wk, dtype=np.float32) * gsk_full[:, None]

    qT = [round_f32r(querys[b].T) for b in range(B)]
    kfT, mb = [], []
    for b in range(B):
        idx = np.flatnonzero(mask[b] != 0)
        cnt = len(idx)
        kc = np.zeros((M, E), np.float32)
        kc[:cnt] = key_feats[b][idx]
        kfT.append(round_f32r(kc.T))
        mbv = np.full((M,), NEG, dtype=np.float32)
        mbv[:cnt] = 0.0
        mb.append(mbv.reshape(n_mt, 128))

    wqT, wkT, wvT, woT, ig2q, ig2k = [], [], [], [], [], []
    for j in range(4):
        dsl = slice(j * DS, (j + 1) * DS)
        wqT.append(round_f32r(Wq_f[dsl].T))
        wkT.append(round_f32r(Wk_f[dsl].T))
        wvT.append(round_f32r(np.asarray(Wv)[dsl].T))
        woT.append(round_f32r(np.asarray(Wo)[:, dsl].T))
        # sumsq compensation: raw sumsq = sum_d (q'_d)^2 / gs_d^2
        ig2q.append(round_f32r((1.0 / gsq_full[dsl] ** 2).reshape(2, 128)))
        ig2k.append(round_f32r((1.0 / gsk_full[dsl] ** 2).reshape(2, 128)))

    in_maps = []
    for cid in range(NCORES):
        b, j = cid // 4, cid % 4
        in_maps.append(
            {
                "qT": qT[b],
                "kfT": kfT[b],
                "wqT": wqT[j],
                "wkT": wkT[j],
                "wvT": wvT[j],
                "woT": woT[j],
                "ig2q": ig2q[j],
                "ig2k": ig2k[j],
                "mbias": mb[b],
            }
        )
    return in_maps


def assemble(results, bo):
    bo = np.asarray(bo, dtype=np.float32)
    out = np.zeros((B, N, D), dtype=np.float32)
    for cid in range(NCORES):
        b = cid // 4
        out[b] += results[cid]["outT"].T
    out += bo
    return out


def kernel(querys, key_feats, mask, Wq, Wk, Wv, gq, gk, Wo, bo):
    n_mt = n_mt_for(mask)
    nc = _get_nc(n_mt)
    in_maps = make_in_maps(querys, key_feats, mask, Wq, Wk, Wv, gq, gk, Wo, bo, n_mt)
    res = run_bass_kernel_spmd(nc, in_maps, list(range(NCORES)))
    return assemble(res.results, bo)


# revision 21
# speedup vs baseline: 1.3946x; 1.1755x over previous
"""CrossAttention Trainium2 kernel (8 NeuronCores).

Reference computation (B=2, N=M=2048, D=1024, H=16, C=64):
    q = rmsnorm(querys @ Wq.T, gq) * C**-0.5       [B,N,D]
    k = rmsnorm(key_feats @ Wk.T, gk)              [B,M,D]
    v = key_feats @ Wv.T                           [B,M,D]
    attn = softmax(mask(q @ k.T per head))         [B,H,N,M]
    out = (attn @ v per head, concat) @ Wo.T + bo  [B,N,D]

Sharding: core = b*4 + j (b in {0,1}; j in {0..3} owns heads 4j..4j+3 = a
256-wide slice of D). Host pre-transposes inputs/weights, folds gq*scale /
gk into Wq / Wk rows, and pre-rounds everything to f32r (fp32 with 11-bit
mantissa -> full PE rate).

v2 structural changes vs v1:
  - Mask compaction: rows with mask==0 contribute exp(-inf)=0 to both the
    softmax denominator and PV, so the host gathers only the valid kf
    columns (per batch), pads to a multiple of 128, and the kernel runs
    with M_pad ~= 1152 instead of 2048.  All M-side work (k/v projection,
    QK, exp, PV, kf DMA) shrinks ~2x.  Padding columns carry bias -1e30
    into the exp -> contribute exactly 0.
  - k and v projections fused over a single kfT stream (halves kf DMA).
  - One fused AllReduce carries both q and k partial sum-of-squares
    (2048 + M_pad floats); its ~27us mesh latency is bridged by a long
    dependency-free dummy-matmul burst that keeps the PE HAM clock-gate
    warm (K=8/8) into attention.
  - Attention is software-pipelined depth-2 per (nbp, h) pass: PE order is
    ... PV(mt-1), QK(mt+1), filler, PV(mt) ... so the PE never stalls on
    the ACT exp (v1 stalled ~0.4us every mt, which kept HAM at K=4/8 =
    1.2 GHz for the whole 314us attention phase).  ACT exp (~1.2us/mt) is
    the pace-setter; PE real work is ~1.0us/mt, padded by a dummy matmul
    (first n-half) or an out-projection matmul pair (second n-half).
  - The out projection (partial over this core's d-slice) is interleaved
    into attention as filler work; the host sums 4 partials per b and
    adds bo.
"""

import numpy as np

import concourse.tile as tile
from concourse import bacc, mybir
from concourse.bass_utils import run_bass_kernel_spmd

B, N, M_FULL, D, H = 2, 2048, 2048, 1024, 16
C = D // H  # 64, head dim
E = D  # input feature dim
EPS = 1e-6
SCALE = C ** (-0.5)
DS = D // 4  # 256, per-core d-slice
NCORES = 8

f32 = mybir.dt.float32
f32r = mybir.dt.float32r
bf16 = mybir.dt.bfloat16
f16 = mybir.dt.float16
AF = mybir.ActivationFunctionType

NEG = -1e30
WARM_MM = 60  # dummy matmuls bridging the AllReduce latency into attention


def round_f32r(x: np.ndarray) -> np.ndarray:
    b = np.ascontiguousarray(x, dtype=np.float32).view(np.uint32)
    b = (b + 0x800) & np.uint32(0xFFFFF000)
    return b.view(np.float32)


def build(n_mt: int):
    M = n_mt * 128
    mblocks = []
    off = 0
    while off < M:
        w = min(512, M - off)
        mblocks.append((off, w))
        off += w

    nc = bacc.Bacc(None, target_bir_lowering=False)

    qT_d = nc.declare_dram_parameter("qT", [E, N], f16, isOutput=False)
    kfT_d = nc.declare_dram_parameter("kfT", [E, M], f16, isOutput=False)
    wqT_d = nc.declare_dram_parameter("wqT", [E, DS], f16, isOutput=False)
    wkT_d = nc.declare_dram_parameter("wkT", [E, DS], f16, isOutput=False)
    wvT_d = nc.declare_dram_parameter("wvT", [E, DS], f16, isOutput=False)
    woT_d = nc.declare_dram_parameter("woT", [DS, D], f16, isOutput=False)
    ig2q_d = nc.declare_dram_parameter("ig2q", [2, 128], f32r, isOutput=False)
    ig2k_d = nc.declare_dram_parameter("ig2k", [2, 128], f32r, isOutput=False)
    mb_d = nc.declare_dram_parameter("mbias", [n_mt, 128], f32, isOutput=False)
    outT_d = nc.declare_dram_parameter("outT", [D, N], f16, isOutput=True)

    with (
        nc.allow_low_precision(reason="f32r matmul operands by design; fp32 PSUM"),
        tile.TileContext(nc) as tc,
    ):
        with (
            tc.tile_pool(name="singles", bufs=1) as singles,
            tc.tile_pool(name="wts", bufs=3) as wts,
            tc.tile_pool(name="blk", bufs=2) as blkpool,
            tc.tile_pool(name="sq", bufs=2) as sqpool,
            tc.tile_pool(name="psb", bufs=3) as ppool,
            tc.tile_pool(name="obuf", bufs=2) as obuf,
            tc.tile_pool(name="osb2", bufs=10) as outbuf,
            tc.tile_pool(name="rdp", bufs=4) as rdp,
            tc.tile_pool(name="small", bufs=2) as small,
            tc.tile_pool(name="dram", bufs=1, space="DRAM") as dram,
        ):
            # ---- constants / small inputs ----
            ones1x64 = singles.tile([1, 64], f32)
            nc.vector.memset(ones1x64, 1.0)
            ones1x128 = singles.tile([1, 128], f32)
            nc.vector.memset(ones1x128, 1.0)
            onesv = singles.tile([128, n_mt * 4], f32)
            nc.vector.memset(onesv, 1.0)
            eps_t = singles.tile([128, 1], f32)
            nc.vector.memset(eps_t, EPS)
            invd_t = singles.tile([128, 1], f32)
            nc.vector.memset(invd_t, 1.0 / D)
            ig2q_sb = singles.tile([128, 2], f32r)
            nc.sync.dma_start(out=ig2q_sb, in_=ig2q_d.rearrange("t p -> p t"))
            ig2k_sb = singles.tile([128, 2], f32r)
            nc.sync.dma_start(out=ig2k_sb, in_=ig2k_d.rearrange("t p -> p t"))
            mb_sb = singles.tile([128, n_mt], f32)
            nc.sync.dma_start(out=mb_sb, in_=mb_d.rearrange("t p -> p t"))

            # weights: wq, wk, wv upfront; wo reuses wq's slot after q proj
            wq_sb = wts.tile([128, 8, DS], f16, tag="w")
            wk_sb = wts.tile([128, 8, DS], f16, tag="w")
            wv_sb = wts.tile([128, 8, DS], f16, tag="w")
            for et in range(8):
                nc.sync.dma_start(out=wq_sb[:, et, :], in_=wqT_d[et * 128 : et * 128 + 128, :])
                nc.scalar.dma_start(out=wk_sb[:, et, :], in_=wkT_d[et * 128 : et * 128 + 128, :])
                nc.scalar.dma_start(out=wv_sb[:, et, :], in_=wvT_d[et * 128 : et * 128 + 128, :])

            # ---- persistent activations ----
            qT = singles.tile([128, 2, 4, 512], f32r)  # [p, dt, nb, n]
            kT = singles.tile([128, 2, M], f32r)  # [p, dt, m]
            v_sb = singles.tile([128, n_mt, 4, C + 1], bf16)  # [m_p, mt, h, c|ones]
            xT = singles.tile([128, 2, 4, 512], f16)  # [p, dt, nb, n]
            nc.vector.tensor_copy(
                v_sb[:, :, :, C], onesv.rearrange("p (a b) -> p a b", a=n_mt)
            )

            ccq_in = dram.tile([2048], f32)
            ccq_out = dram.tile([2048], f32)
            cck_in = dram.tile([M], f32)
            cck_out = dram.tile([M], f32)
            rsq_dram = dram.tile([2048], f32)

            with (
                tc.tile_pool(name="projps", bufs=2, space="PSUM") as projps,
                tc.tile_pool(name="vps", bufs=2, space="PSUM") as vps,
                tc.tile_pool(name="ssps", bufs=2, space="PSUM") as ssps,
            ):
                # ---- fused k+v projection over one kfT stream ----
                for moff, w in mblocks:
                    blk = blkpool.tile([128, 8, 512], f16, tag="blk")
                    for et in range(8):
                        nc.scalar.dma_start(
                            out=blk[:, et, 0:w],
                            in_=kfT_d[et * 128 : et * 128 + 128, moff : moff + w],
                        )
                    ss_ps = ssps.tile([1, 512], f32, tag="ss")
                    for dt in range(2):
                        ps = projps.tile([128, 512], f32, tag="proj")
                        for et in range(8):
                            nc.tensor.matmul(
                                ps[:, 0:w],
                                wk_sb[:, et, dt * 128 : dt * 128 + 128],
                                blk[:, et, 0:w],
                                start=(et == 0),
                                stop=(et == 7),
                            )
                        nc.vector.tensor_copy(kT[:, dt, moff : moff + w], ps[:, 0:w])
                        sq = sqpool.tile([128, 512], f32r, tag="sq")
                        nc.vector.tensor_mul(
                            sq[:, 0:w],
                            kT[:, dt, moff : moff + w],
                            kT[:, dt, moff : moff + w],
                        )
                        nc.tensor.matmul(
                            ss_ps[:, 0:w],
                            ig2k_sb[:, dt : dt + 1],
                            sq[:, 0:w],
                            start=(dt == 0),
                            stop=(dt == 1),
                            skip_group_check=True,
                        )
                    ss_sb = small.tile([1, 512], f32, tag="ss_sb")
                    nc.scalar.copy(ss_sb[:, 0:w], ss_ps[:, 0:w])
                    nc.sync.dma_start(
                        out=cck_in[moff : moff + w].rearrange(
                            "(a n) -> a n", a=1
                        ),
                        in_=ss_sb[:, 0:w],
                    )
                    # v from the same resident block
                    for ct in range(w // 128):
                        mtg = moff // 128 + ct
                        psv = vps.tile([128, 256], f32, tag="v")
                        for et in range(8):
                            nc.tensor.matmul(
                                psv,
                                blk[:, et, ct * 128 : ct * 128 + 128],
                                wv_sb[:, et, :],
                                start=(et == 0),
                                stop=(et == 7),
                            )
                        nc.vector.tensor_copy(
                            v_sb[:, mtg, :, 0:C],
                            psv.rearrange("p (h c) -> p h c", c=C),
                        )

                nc.gpsimd.collective_compute(
                    "AllReduce",
                    mybir.AluOpType.add,
                    replica_groups=[[0, 1, 2, 3], [4, 5, 6, 7]],
                    ins=[cck_in.opt()],
                    outs=[cck_out.opt()],
                )

                # ---- q projection: qT[dt, nb] = Wq'^T-slice @ q-block ----
                for nb in range(4):
                    blk = blkpool.tile([128, 8, 512], f16, tag="blk")
                    for et in range(8):
                        nc.sync.dma_start(
                            out=blk[:, et, :],
                            in_=qT_d[et * 128 : et * 128 + 128, nb * 512 : nb * 512 + 512],
                        )
                    ss_ps = ssps.tile([1, 512], f32, tag="ss")
                    for dt in range(2):
                        ps = projps.tile([128, 512], f32, tag="proj")
                        for et in range(8):
                            nc.tensor.matmul(
                                ps,
                                wq_sb[:, et, dt * 128 : dt * 128 + 128],
                                blk[:, et, :],
                                start=(et == 0),
                                stop=(et == 7),
                            )
                        nc.vector.tensor_copy(qT[:, dt, nb, :], ps)
                        sq = sqpool.tile([128, 512], f32r, tag="sq")
                        nc.vector.tensor_mul(sq, qT[:, dt, nb, :], qT[:, dt, nb, :])
                        nc.tensor.matmul(
                            ss_ps,
                            ig2q_sb[:, dt : dt + 1],
                            sq,
                            start=(dt == 0),
                            stop=(dt == 1),
                            skip_group_check=True,
                        )
                    ss_sb = small.tile([1, 512], f32, tag="ss_sb")
                    nc.scalar.copy(ss_sb, ss_ps)
                    nc.sync.dma_start(
                        out=ccq_in[nb * 512 : nb * 512 + 512].rearrange(
                            "(a n) -> a n", a=1
                        ),
                        in_=ss_sb,
                    )

                # AR_q fires while the kv projection runs; AR_k pipelines
                # behind it on the CC queue.
                nc.gpsimd.collective_compute(
                    "AllReduce",
                    mybir.AluOpType.add,
                    replica_groups=[[0, 1, 2, 3], [4, 5, 6, 7]],
                    ins=[ccq_in.opt()],
                    outs=[ccq_out.opt()],
                )

                # wo weight load (DMA only; hides under the AllReduce)
                wo_sb = wts.tile([128, 2, D], f16, tag="w")
                for dc in range(2):
                    nc.sync.dma_start(
                        out=wo_sb[:, dc, :], in_=woT_d[dc * 128 : dc * 128 + 128, :]
                    )

                # ---- warm burst: dependency-free matmuls spanning the
                # AllReduce latency so the PE HAM clock-gate stays at 8/8
                # when attention starts (a PE-idle window here would drop it
                # to 4/8 and the attention stream would start cold) ----
                warm = projps.tile([128, 512], f32, tag="proj")
                for i in range(WARM_MM):
                    nc.tensor.matmul(
                        warm,
                        kT[:, 0, 0:128],
                        kT[:, 0, 512:1024],
                        start=(i == 0),
                        stop=(i == WARM_MM - 1),
                        skip_group_check=True,
                    )
                warm_sink = small.tile([1, 512], f32, tag="rd")
                nc.vector.tensor_copy(warm_sink, warm[0:1, :])

                # ---- rstd_k: [128, n_mt] lane-parallel; feeds exp scale ----
                ss128 = small.tile([128, n_mt], f32, tag="ssk")
                nc.sync.dma_start(
                    out=ss128, in_=cck_out.rearrange("(t p) -> p t", p=128)
                )
                stdk = small.tile([128, n_mt], f32, tag="stdk")
                nc.scalar.activation(stdk, ss128, AF.Sqrt, bias=eps_t, scale=invd_t)
                rstdk = singles.tile([128, n_mt], f32)
                nc.vector.reciprocal_approx_fast(out=rstdk, in_=stdk)

                # ---- rstd_q: lane-parallel [128, 16], then a DRAM bounce to
                # the [1, 2048] row layout the bcast outer-product wants
                # (single-lane sqrt/recip on [1, 2048] costs ~5us; this ~1us)
                ssq128 = small.tile([128, 16], f32, tag="ssq")
                nc.sync.dma_start(
                    out=ssq128, in_=ccq_out.rearrange("(t p) -> p t", p=128)
                )
                stdq = small.tile([128, 16], f32, tag="stdq")
                nc.scalar.activation(stdq, ssq128, AF.Sqrt, bias=eps_t, scale=invd_t)
                rsq128 = small.tile([128, 16], f32, tag="rsq")
                nc.vector.reciprocal_approx_fast(out=rsq128, in_=stdq)
                nc.sync.dma_start(
                    out=rsq_dram.rearrange("(t p) -> p t", p=128), in_=rsq128
                )
                rs_row = singles.tile([1, 2048], f32)
                nc.sync.dma_start(
                    out=rs_row, in_=rsq_dram.rearrange("(a n) -> a n", a=1)
                )
                # q finalize: qT[d, n] *= rstd_q[n] via ones outer-product bcast
                for nb in range(4):
                    bcq = projps.tile([128, 512], f32, tag="proj")
                    nc.tensor.matmul(
                        bcq,
                        ones1x128,
                        rs_row[:, nb * 512 : nb * 512 + 512],
                        start=True,
                        stop=True,
                    )
                    for dt in range(2):
                        nc.vector.tensor_mul(qT[:, dt, nb, :], qT[:, dt, nb, :], bcq)

            # ---- attention: flat stream over g = (pass, mt), pipelined
            # depth-2 ACROSS pass boundaries.  PE order:
            #   ... PV(g-1), QK(g+1), filler, PV(g), QK(g+2), ...
            # ACT order: exp(0), exp(1), ...  The PE never waits on an exp
            # (QK(g+1)'s s2 slot was freed by exp(g-1), one full exp ago) so
            # the HAM clock-gate stays warm.
            # PSUM banks: s2 2x2 + o2 2 + filler/bc/dummy 2x1 = 8.
            with (
                tc.tile_pool(name="sps", bufs=2, space="PSUM") as spool,
                tc.tile_pool(name="ops", bufs=1, space="PSUM") as opool,
                tc.tile_pool(name="fps", bufs=2, space="PSUM") as fpool,
            ):
                # passes are (head-pair hp, nb): the two heads of a pair sit
                # at partitions 0:64 / 64:128 of dt=hp, so their QK matmuls
                # run CONCURRENTLY as row-group tiles (K=64 each) -- one
                # 512-cycle stream produces both heads' s2.  nb-major order
                # so each nb's out-projection unlocks early as PE filler.
                passes = [(hp, nb) for nb in range(4) for hp in range(2)]
                G = len(passes) * n_mt
                s2g, p2g, o2cur = {}, {}, {}
                filler = []
                prev = [None]

                def emit_dummy():
                    dum = fpool.tile([128, 512], f32, tag="f")
                    nc.tensor.matmul(
                        dum, kT[:, 0, 0:128], kT[:, 0, 512:1024],
                        start=True, stop=True, skip_group_check=True,
                    )

                def make_outproj(nb):
                    """one closure per ot psum tile: 2 matmuls + copy + dma"""
                    fns = []
                    for ot in range(8):
                        def f(nb=nb, ot=ot):
                            ps = fpool.tile([128, 512], f32, tag="f")
                            for dc in range(2):
                                nc.tensor.matmul(
                                    ps,
                                    wo_sb[:, dc, ot * 128 : ot * 128 + 128],
                                    xT[:, dc, nb, :],
                                    start=(dc == 0),
                                    stop=(dc == 1),
                                )
                            out_sb = outbuf.tile([128, 512], f16, tag="osb")
                            nc.vector.tensor_copy(out_sb, ps)
                            nc.sync.dma_start(
                                out=outT_d[
                                    ot * 128 : ot * 128 + 128,
                                    nb * 512 : nb * 512 + 512,
                                ],
                                in_=out_sb,
                            )
                        fns.append(f)
                    return fns

                def emit_normalize(state):
                    """bc outer-products + muls for a pass whose DVE recips are
                    done by now (emitted one pass late to keep PE gapless)."""
                    hp_, nb_, oo_sb, rds_ = state
                    for i in range(2):
                        bc = fpool.tile([128, 512], f32, tag="f")
                        nc.tensor.matmul(
                            bc[0:C, :], ones1x64, rds_[i], start=True, stop=True
                        )
                        nc.vector.tensor_mul(
                            xT[i * 64 : i * 64 + C, hp_, nb_, :],
                            oo_sb[0:C, i, :],
                            bc[0:C, :],
                        )

                def emit_qk(g):
                    (hp, nb), mt = passes[g // n_mt], g % n_mt
                    s2 = spool.tile([128, 2, 512], f32, tag="s2")
                    for i in range(2):
                        nc.tensor.matmul(
                            s2[:, i, :],
                            kT[i * 64 : i * 64 + 64, hp, mt * 128 : mt * 128 + 128],
                            qT[i * 64 : i * 64 + 64, hp, nb, :],
                            start=True,
                            stop=True,
                        )
                    s2g[g] = s2

                def emit_exp(g):
                    mt = g % n_mt
                    p2 = ppool.tile([128, 2, 512], bf16, tag="p")
                    nc.scalar.activation(
                        p2, s2g.pop(g), AF.Exp,
                        bias=mb_sb[:, mt : mt + 1],
                        scale=rstdk[:, mt : mt + 1],
                    )
                    p2g[g] = p2

                def emit_pv(g):
                    pi, mt = g // n_mt, g % n_mt
                    hp, nb = passes[pi]
                    if mt == 0:
                        o2_t = opool.tile([C + 1, 2, 512], f32, tag="o2")
                        o2cur[pi] = o2_t
                    o2 = o2cur[pi]
                    p2 = p2g.pop(g)
                    for i in range(2):
                        nc.tensor.matmul(
                            o2[:, i, :],
                            v_sb[:, mt, 2 * hp + i, :],
                            p2[:, i, :],
                            start=(mt == 0),
                            stop=(mt == n_mt - 1),
                            skip_group_check=True,
                        )
                    if mt == n_mt - 1:
                        # pass drain: free o2 with one DVE copy; recips; the
                        # PREVIOUS pass's normalize (its recips are done)
                        o_sb = obuf.tile([C + 1, 2, 512], f32, tag="osb")
                        nc.vector.tensor_copy(o_sb, o2cur.pop(pi))
                        rds = []
                        for i in range(2):
                            den = rdp.tile([1, 512], f32, tag="den")
                            nc.vector.tensor_copy(den, o_sb[C : C + 1, i, :])
                            rd = rdp.tile([1, 512], f32, tag="rd")
                            nc.vector.reciprocal_approx_fast(out=rd, in_=den)
                            rds.append(rd)
                        if prev[0] is not None:
                            emit_normalize(prev[0])
                            if prev[0][0] == 1 and prev[0][1] < 3:
                                # both head-pairs of nb done -> its out
                                # projection becomes PE filler
                                filler.extend(make_outproj(prev[0][1]))
                        prev[0] = (hp, nb, o_sb, rds)

                emit_qk(0)
                emit_qk(1)
                rewarm = fpool.tile([128, 512], f32, tag="f")
                for i in range(40):
                    nc.tensor.matmul(
                        rewarm, kT[:, 0, 0:128], kT[:, 0, 512:1024],
                        start=(i == 0), stop=(i == 39), skip_group_check=True,
                    )
                emit_exp(0)
                for _ in range(2):
                    emit_dummy()
                for g in range(1, G):
                    emit_pv(g - 1)
                    if g + 1 < G:
                        emit_qk(g + 1)
                    if filler:
                        filler.pop(0)()
                    else:
                        emit_dummy()
                        emit_dummy()
                        if g < 3 * n_mt:
                            emit_dummy()
                    emit_exp(g)
                emit_pv(G - 1)

                emit_normalize(prev[0])
                # tail: whatever filler wasn't consumed + last nb's out proj
                for f in filler:
                    f()
                for f in make_outproj(3):
                    f()

    nc.finalize()
    return nc


_NC_CACHE = {}


def _get_nc(n_mt: int):
    if n_mt not in _NC_CACHE:
        _NC_CACHE[n_mt] = build(n_mt)
    return _NC_CACHE[n_mt]


def n_mt_for(mask) -> int:
    mask = np.asarray(mask)
    cnt = int(max((mask[b] != 0).sum() for b in range(B)))
    return max(1, (cnt + 127) // 128)


def make_in_maps(querys, key_feats, mask, Wq, Wk, Wv, gq, gk, Wo, bo, n_mt):
    M = n_mt * 128
    querys = np.asarray(querys, dtype=np.float32)
    key_feats = np.asarray(key_feats, dtype=np.float32)
    mask = np.asarray(mask)
    gq = np.asarray(gq, dtype=np.float32)
    gk = np.asarray(gk, dtype=np.float32)

    gsq_full = gq * np.float32(SCALE)  # folded into Wq rows
    gsk_full = gk.astype(np.float32)  # folded into Wk rows
    Wq_f = np.asarray(Wq, dtype=np.float32) * gsq_full[:, None]
    Wk_f = np.asarray(Wk, dtype=np.float32) * gsk_full[:, None]

    qT = [np.ascontiguousarray(querys[b].T).astype(np.float16) for b in range(B)]
    kfT, mb = [], []
    for b in range(B):
        idx = np.flatnonzero(mask[b] != 0)
        cnt = len(idx)
        kc = np.zeros((M, E), np.float32)
        kc[:cnt] = key_feats[b][idx]
        kfT.append(np.ascontiguousarray(kc.T).astype(np.float16))
        mbv = np.full((M,), NEG, dtype=np.float32)
        mbv[:cnt] = 0.0
        mb.append(mbv.reshape(n_mt, 128))

    wqT, wkT, wvT, woT, ig2q, ig2k = [], [], [], [], [], []
    for j in range(4):
        dsl = slice(j * DS, (j + 1) * DS)
        wqT.append(np.ascontiguousarray(Wq_f[dsl].T).astype(np.float16))
        wkT.append(np.ascontiguousarray(Wk_f[dsl].T).astype(np.float16))
        wvT.append(np.ascontiguousarray(np.asarray(Wv, dtype=np.float32)[dsl].T).astype(np.float16))
        woT.append(np.ascontiguousarray(np.asarray(Wo, dtype=np.float32)[:, dsl].T).astype(np.float16))
        # sumsq compensation: raw sumsq = sum_d (q'_d)^2 / gs_d^2
        ig2q.append(round_f32r((1.0 / gsq_full[dsl] ** 2).reshape(2, 128)))
        ig2k.append(round_f32r((1.0 / gsk_full[dsl] ** 2).reshape(2, 128)))

    in_maps = []
    for cid in range(NCORES):
        b, j = cid // 4, cid % 4
        in_maps.append(
            {
                "qT": qT[b],
                "kfT": kfT[b],
                "wqT": wqT[j],
                "wkT": wkT[j],
                "wvT": wvT[j],
                "woT": woT[j],
                "ig2q": ig2q[j],
                "ig2k": ig2k[j],
                "mbias": mb[b],
            }
        )
    return in_maps


def assemble(results, bo):
    bo = np.asarray(bo, dtype=np.float32)
    out = np.zeros((B, N, D), dtype=np.float32)
    for cid in range(NCORES):
        b = cid // 4
        out[b] += results[cid]["outT"].T.astype(np.float32)
    out += bo
    return out


def kernel(querys, key_feats, mask, Wq, Wk, Wv, gq, gk, Wo, bo):
    n_mt = n_mt_for(mask)
    nc = _get_nc(n_mt)
    in_maps = make_in_maps(querys, key_feats, mask, Wq, Wk, Wv, gq, gk, Wo, bo, n_mt)
    res = run_bass_kernel_spmd(nc, in_maps, list(range(NCORES)))
    return assemble(res.results, bo)
